# revision 5
# baseline (speedup 1.0000x reference)
"""Trainium2 Bass kernel for nn_PrimalDual (primal-dual multi-label segmentation).

Strategy:
  - Shard the image rows (h) across 8 cores; each core owns ROWS=48 output rows
    plus G=repeats ghost rows on each side computed redundantly, so no
    inter-core communication is needed (the ghost region shrinks by one row per
    iteration and is exactly exhausted after `repeats` iterations).
  - All state lives in SBUF for the whole solve: u (f32), ubar/p1/p2/p3 (f16),
    s1/s2/mu1/mu2 (f16, the proj=78-sized dual variables).
  - Layout: partition q in [0,128) holds image columns w = C*q + c, c in [0,C),
    C = W/128; free dims are (h_local, c, z|proj).
  - The einsum mu->z and the interval sums z->proj are done with segmented
    scans (tensor_tensor_scan) plus grouped strided subtract ops; everything
    else is pointwise chains on DVE/ACT.
"""

import numpy as np
from contextlib import ExitStack

import concourse.bass as bass
import concourse.tile as tile
from concourse import bacc, mybir
from concourse.bass_utils import run_bass_kernel_spmd

F16 = mybir.dt.float16
U8 = mybir.dt.uint8
F32 = mybir.dt.float32
AF = mybir.ActivationFunctionType
OP = mybir.AluOpType

# problem geometry (from spec; patchable for small-config sim tests)
CFG = dict(H=384, W=384, L=12, NCORES=8, P=128)

AB = 8   # A/C-phase row-block
BB = 8   # B-phase row-block

_HALF_PI = 1.5707963267948966


def flat(ap):
    nd = len(ap.shape)
    if nd == 2:
        return ap
    names = " ".join(f"d{i}" for i in range(nd - 1))
    return ap.rearrange(f"p {names} -> p ({names})")


def _register_consts(nc, values):
    for v in values:
        v = float(v)
        if (mybir.dt.float32, v) in nc.const_aps.aps:
            continue
        t = nc.alloc_sbuf_tensor(f"constf32-{len(nc.const_aps.aps)}", [128, 1], F32)
        nc.gpsimd.memset(t.ap(), v)
        nc.const_aps.aps[(mybir.dt.float32, v)] = t.ap()
    nc.all_engine_barrier()


def _blocks(lo, hi, step):
    out = []
    r = lo
    while r < hi:
        out.append((r, min(r + step, hi)))
        r = out[-1][1]
    return out


def build_program(lmbda, nu, repeats, l, cfg=None):
    cfg = cfg or CFG
    H, W, L, NCORES, P = cfg["H"], cfg["W"], cfg["L"], cfg["NCORES"], cfg["P"]
    assert L == l
    assert W % P == 0
    C = W // P
    ROWS = H // NCORES
    G = repeats
    SLAB = ROWS + 2 * G
    PROJ = l * (l + 1) // 2

    sigmap = 1.0 / (3.0 + l)
    tauu = 1.0 / 6.0
    tau_mu = 1.0 / (2.0 + PROJ / 4.0)
    lmbda = float(lmbda)
    nu = float(nu)
    sql = float(np.sqrt(lmbda))
    kl = [(z + 1) / l for z in range(l)]

    # run offsets: off(k1) = start index of the k1-run in p-order (k1-major)
    off = [0] * (l + 1)
    for k1 in range(l):
        off[k1 + 1] = off[k1] + (l - k1)

    nc = bacc.Bacc("TRN2", target_bir_lowering=False, debug=False,
                   num_devices=NCORES)
    _register_consts(nc, [sql * k for k in kl] + [2.0 / 3.0, _HALF_PI])

    f_in = nc.dram_tensor("f_in", [P, SLAB * C], F32, kind="ExternalInput")
    mA_in = nc.dram_tensor("mA_in", [P, SLAB], F16, kind="ExternalInput")
    mC_in = nc.dram_tensor("mC_in", [P, SLAB], F16, kind="ExternalInput")
    wm_in = nc.dram_tensor("wm_in", [P, 2], F32, kind="ExternalInput")
    u_out = nc.dram_tensor("u_out", [P, ROWS * C * L], F32, kind="ExternalOutput")

    with tile.TileContext(nc) as tc, ExitStack() as ctx, \
            nc.allow_low_precision(reason="f16 state by design"):
        V = nc.vector
        S = nc.scalar

        st = ctx.enter_context(tc.tile_pool(name="state", bufs=1))
        u = st.tile([P, SLAB, C, L], F32)
        ubar = st.tile([P, SLAB, C, L], F16)
        p1 = st.tile([P, SLAB, C, L], F16)
        p2 = st.tile([P, SLAB, C, L], F16)
        p3 = st.tile([P, SLAB, C, L], F16)
        s1 = st.tile([P, SLAB, C, PROJ], F16)
        s2 = st.tile([P, SLAB, C, PROJ], F16)
        mu1 = st.tile([P, SLAB, C, PROJ], F16)
        mu2 = st.tile([P, SLAB, C, PROJ], F16)
        ld2 = st.tile([P, SLAB, C, L], F16)
        fsb = st.tile([P, SLAB, C], F32)
        mA = st.tile([P, SLAB], F16)
        mC = st.tile([P, SLAB], F16)
        zmb = st.tile([P, AB, C, L], F16)     # z-segment mask block (0 at z=0)
        pmb = st.tile([P, AB, C, PROJ], F16)  # proj-segment mask block
        # w-shift staging (cross-partition neighbours via DMA)
        wm = st.tile([P, 2], F32)             # [wA, -wA] per-partition
        wsu = st.tile([P, SLAB, L], F16)      # ubar[q+1, c=0] staged at q
        wsp = st.tile([P, SLAB, L], F16)      # p2[q-1, c=C-1] staged at q

        at_ = ctx.enter_context(tc.tile_pool(name="atemp", bufs=2))
        bt_ = ctx.enter_context(tc.tile_pool(name="btemp", bufs=2))

        def atile(tag, dt=F16):
            return at_.tile([P, AB, C, L], dt, tag=tag, name=tag)

        def btile(tag, dt=F16):
            return bt_.tile([P, BB, C, PROJ], dt, tag=tag, name=tag)

        def bcast_h(m, lo, hi, last):
            return m[:, lo:hi].unsqueeze(2).unsqueeze(3).broadcast_to(
                [P, hi - lo, C, last])

        # ---------------- init ----------------
        nc.sync.dma_start(flat(fsb[:]), f_in.ap())
        nc.sync.dma_start(mA[:], mA_in.ap())
        nc.sync.dma_start(mC[:], mC_in.ap())
        nc.sync.dma_start(wm[:], wm_in.ap())
        fb = fsb[:].unsqueeze(3).broadcast_to([P, SLAB, C, L])
        V.tensor_copy(u[:], fb)
        V.tensor_copy(ubar[:], fb)
        for z in range(L):
            S.activation(ld2[:, :, :, z:z + 1], fsb[:].unsqueeze(3),
                         AF.Square, scale=-sql, bias=sql * kl[z])
        for t in (p1, p2, p3, s1, s2, mu1, mu2):
            V.memset(t[:], 0.0)
        V.memset(zmb[:], 1.0)
        V.memset(zmb[:, :, :, 0:1], 0.0)
        V.memset(pmb[:], 1.0)
        V.memset(pmb[:, :, :, 0:1], 0.0)
        V.memset(wsu[:], 0.0)
        V.memset(wsp[:], 0.0)

        # ---------------- iterations ----------------
        for it in range(repeats):
            lo, hi = it + 1, SLAB - 1 - it
            if NCORES == 1:
                lo, hi = G, G + ROWS  # no ghost shrink needed, masks do edges
            # A/B phases need one extra row above: clipping at row r consumes
            # the same-iteration parabola output at r-1.
            ablo = max(lo - 1, 0)

            # stage w-neighbours for the whole row range
            nc.sync.dma_start(wsu[0:P - 1, ablo:hi].unsqueeze(2),
                              ubar[1:P, ablo:hi, 0:1])
            # ubar[w+1] for w=W-1 is "replicate last": A-mask kills u2 there,
            # but keep the stale zeros in wsu row P-1 (never read: see memset).

            # ======== A phase: parabola ========
            for (alo, ahi) in _blocks(ablo, hi, AB):
                R = ahi - alo

                def asl(tl, s=0, e=None):
                    return tl[:, alo + s: ahi + (e or 0)]

                u1 = atile("u1")
                u2 = atile("u2")
                u3 = atile("u3")
                # u1 = (ubar[r+1]-ubar[r]) * A
                V.tensor_tensor(u1[:, :R], ubar[:, alo + 1:ahi + 1],
                                ubar[:, alo:ahi], op=OP.subtract)
                V.tensor_tensor(u1[:, :R], u1[:, :R],
                                bcast_h(mA, alo, ahi, L), op=OP.mult)
                V.scalar_tensor_tensor(u1[:, :R], u1[:, :R], sigmap,
                                       p1[:, alo:ahi], op0=OP.mult, op1=OP.add)
                # u2 = (ubar[w+1]-ubar[w]); w=W-1 -> 0
                if C > 1:
                    V.tensor_tensor(u2[:, :R, 0:C - 1],
                                    ubar[:, alo:ahi, 1:C],
                                    ubar[:, alo:ahi, 0:C - 1], op=OP.subtract)
                V.scalar_tensor_tensor(u2[:, :R, C - 1:C],
                                       ubar[:, alo:ahi, C - 1:C],
                                       wm[:, 1:2], wsu[:, alo:ahi].unsqueeze(2),
                                       op0=OP.mult, op1=OP.add)
                V.scalar_tensor_tensor(u2[:, :R], u2[:, :R], sigmap,
                                       p2[:, alo:ahi], op0=OP.mult, op1=OP.add)
                # u3 = dz(ubar); z=L-1 -> 0
                V.tensor_tensor(u3[:, :R, :, 0:L - 1],
                                ubar[:, alo:ahi, :, 1:L],
                                ubar[:, alo:ahi, :, 0:L - 1], op=OP.subtract)
                V.memset(u3[:, :R, :, L - 1:L], 0.0)
                V.scalar_tensor_tensor(u3[:, :R], u3[:, :R], sigmap,
                                       p3[:, alo:ahi], op0=OP.mult, op1=OP.add)

                # fold sigmap * mu1sum into u1 (and mu2sum into u2)
                for (mus, uacc) in ((mu1, u1), (mu2, u2)):
                    csp = at_.tile([P, AB * C * PROJ], F16, tag="csp", name="csp")
                    V.tensor_tensor_scan(
                        csp[:, :R * C * PROJ], flat(pmb[:, :R]),
                        flat(mus[:, alo:ahi]), 0.0, op0=OP.mult, op1=OP.add)
                    cs4 = csp[:, :R * C * PROJ].rearrange(
                        "p (r c j) -> p r c j", r=R, c=C, j=PROJ)
                    # += sigmap * cs[off(k1+1)-1] for k1 <= z
                    for k1 in range(l):
                        V.scalar_tensor_tensor(
                            uacc[:, :R, :, k1:L],
                            cs4[:, :, :, off[k1 + 1] - 1:off[k1 + 1]]
                            .broadcast_to([P, R, C, L - k1]),
                            sigmap, uacc[:, :R, :, k1:L],
                            op0=OP.mult, op1=OP.add)
                    # -= sigmap * cs[off(k1) + z-k1-1] for z >= k1 (skip z=k1==0)
                    for k1 in range(l):
                        z0 = max(k1, 1)
                        a = off[k1] + z0 - k1 - 1
                        V.scalar_tensor_tensor(
                            uacc[:, :R, :, z0:L],
                            cs4[:, :, :, a:a + (L - z0)],
                            -sigmap, uacc[:, :R, :, z0:L],
                            op0=OP.mult, op1=OP.add)

                # cubic solve
                q2 = atile("q2")
                tq = atile("tq")
                V.tensor_tensor(q2[:, :R], u1[:, :R], u1[:, :R], op=OP.mult)
                V.tensor_tensor(tq[:, :R], u2[:, :R], u2[:, :R], op=OP.mult)
                V.tensor_tensor(q2[:, :R], q2[:, :R], tq[:, :R], op=OP.add)
                bv = atile("dd")
                V.scalar_tensor_tensor(bv[:, :R], q2[:, :R], 0.25,
                                       ld2[:, alo:ahi], op0=OP.mult,
                                       op1=OP.subtract)
                msk = atile("msk", U8)
                V.tensor_tensor(msk[:, :R], u3[:, :R], bv[:, :R], op=OP.is_lt)
                bq = atile("bq")
                V.tensor_tensor(bq[:, :R], u3[:, :R], ld2[:, alo:ahi], op=OP.add)
                V.tensor_scalar(bq[:, :R], bq[:, :R], -1.0 / 3.0, 2.0 / 3.0,
                                op0=OP.mult, op1=OP.add)
                b3 = atile("b3")
                V.tensor_tensor(b3[:, :R], bq[:, :R], bq[:, :R], op=OP.mult)
                V.tensor_tensor(b3[:, :R], b3[:, :R], bq[:, :R], op=OP.mult)
                dd = atile("dd")
                V.scalar_tensor_tensor(dd[:, :R], q2[:, :R], 0.25, b3[:, :R],
                                       op0=OP.mult, op1=OP.add)
                dneg = atile("dneg", U8)
                V.tensor_scalar(dneg[:, :R], dd[:, :R], 0.0, None, op0=OP.is_lt)
                norm = atile("norm")
                S.activation(norm[:, :R], q2[:, :R], AF.Sqrt)
                # c = cbrt(0.5*norm + sqrt(max(d,0)))
                V.tensor_scalar(dd[:, :R], dd[:, :R], 0.0, None, op0=OP.max)
                sq = atile("sq")
                S.activation(sq[:, :R], dd[:, :R], AF.Sqrt)
                V.scalar_tensor_tensor(sq[:, :R], norm[:, :R], 0.5, sq[:, :R],
                                       op0=OP.mult, op1=OP.add)
                cc = atile("cc")
                S.activation(cc[:, :R], sq[:, :R], AF.Ln)
                S.activation(cc[:, :R], cc[:, :R], AF.Exp, scale=1.0 / 3.0)
                rc = atile("rc")
                V.reciprocal(rc[:, :R], cc[:, :R])
                vv = atile("vv")
                V.tensor_tensor(vv[:, :R], bq[:, :R], rc[:, :R], op=OP.mult)
                V.tensor_tensor(vv[:, :R], cc[:, :R], vv[:, :R], op=OP.subtract)
                # trig branch: v = 2*sb*cos(arccos(ratio)/3), ratio=norm/(2*sb3)
                sb3 = atile("sb3")
                V.tensor_scalar(sb3[:, :R], b3[:, :R], 0.0, None, op0=OP.min)
                S.activation(sb3[:, :R], sb3[:, :R], AF.Sqrt, scale=-4.0)
                V.reciprocal(sb3[:, :R], sb3[:, :R])
                rat = atile("sq")
                V.tensor_tensor(rat[:, :R], norm[:, :R], sb3[:, :R], op=OP.mult)
                V.tensor_scalar(rat[:, :R], rat[:, :R], 0.0, 1.0,
                                op0=OP.max, op1=OP.min)
                # t = sqrt((1-r)/(1+r)); theta = 2*atan(t)
                den = atile("dd")
                V.tensor_scalar(den[:, :R], rat[:, :R], 1.0, None, op0=OP.add)
                V.reciprocal(den[:, :R], den[:, :R])
                V.tensor_scalar(rat[:, :R], rat[:, :R], -1.0, 1.0,
                                op0=OP.mult, op1=OP.add)
                V.tensor_tensor(rat[:, :R], rat[:, :R], den[:, :R], op=OP.mult)
                S.activation(rat[:, :R], rat[:, :R], AF.Sqrt)
                S.activation(rat[:, :R], rat[:, :R], AF.Arctan)
                # v_s = sin(pi/2 - (2/3)atan) = cos(theta/3)
                S.activation(rat[:, :R], rat[:, :R], AF.Sin,
                             scale=-2.0 / 3.0, bias=_HALF_PI)
                sb2 = atile("b3")
                V.tensor_scalar(sb2[:, :R], bq[:, :R], 0.0, None, op0=OP.min)
                S.activation(sb2[:, :R], sb2[:, :R], AF.Sqrt, scale=-4.0)
                V.tensor_tensor(sb2[:, :R], sb2[:, :R], rat[:, :R], op=OP.mult)
                V.copy_predicated(vv[:, :R], dneg[:, :R], sb2[:, :R])
                # scale = 2*v/norm, guarded by norm>0
                V.reciprocal(norm[:, :R], norm[:, :R])
                V.scalar_tensor_tensor(vv[:, :R], vv[:, :R], 2.0, norm[:, :R],
                                       op0=OP.mult, op1=OP.mult)
                nzm = atile("nzm", U8)
                V.tensor_scalar(nzm[:, :R], q2[:, :R], 0.0, None, op0=OP.is_gt)
                V.tensor_tensor(nzm[:, :R], nzm[:, :R], msk[:, :R],
                                op=OP.logical_and)
                # p1,p2 update (in place)
                gu = atile("cc")
                V.tensor_tensor(gu[:, :R], vv[:, :R], u1[:, :R], op=OP.mult)
                V.tensor_copy(p1[:, alo:ahi], u1[:, :R])
                V.copy_predicated(p1[:, alo:ahi], nzm[:, :R], gu[:, :R])
                V.tensor_tensor(gu[:, :R], vv[:, :R], u2[:, :R], op=OP.mult)
                V.tensor_copy(p2[:, alo:ahi], u2[:, :R])
                V.copy_predicated(p2[:, alo:ahi], nzm[:, :R], gu[:, :R])
                # p3 = where(mask, 0.25*(p1n^2+p2n^2) - ld2, u3)
                tq2 = atile("tq")
                V.tensor_tensor(q2[:, :R], p1[:, alo:ahi], p1[:, alo:ahi],
                                op=OP.mult)
                V.tensor_tensor(tq2[:, :R], p2[:, alo:ahi], p2[:, alo:ahi],
                                op=OP.mult)
                V.tensor_tensor(q2[:, :R], q2[:, :R], tq2[:, :R], op=OP.add)
                V.scalar_tensor_tensor(q2[:, :R], q2[:, :R], 0.25,
                                       ld2[:, alo:ahi], op0=OP.mult,
                                       op1=OP.subtract)
                V.tensor_copy(p3[:, alo:ahi], u3[:, :R])
                V.copy_predicated(p3[:, alo:ahi], msk[:, :R], q2[:, :R])

            # ======== B phase: interval sums, mu update, l2proj ========
            for (blo, bhi) in _blocks(ablo, hi, BB):
                R = bhi - blo
                zc1 = bt_.tile([P, BB * C * L], F16, tag="zc1", name="zc1")
                zc2 = bt_.tile([P, BB * C * L], F16, tag="zc2", name="zc2")
                pt = bt_.tile([P, BB, C, L], F16, tag="pt", name="pt")
                for (pn, zc) in ((p1, zc1), (p2, zc2)):
                    V.tensor_scalar_mul(pt[:, :R], pn[:, blo:bhi], tau_mu)
                    V.tensor_tensor_scan(
                        zc[:, :R * C * L], flat(zmb[:, :R]), flat(pt[:, :R]),
                        0.0, op0=OP.mult, op1=OP.add)
                for (sx, mux, zc) in ((s1, mu1, zc1), (s2, mu2, zc2)):
                    zc4 = zc[:, :R * C * L].rearrange(
                        "p (r c z) -> p r c z", r=R, c=C, z=L)
                    # delta = tau*(s - t1): build t1tau into dl then finish
                    dl = btile("dl")
                    for k1 in range(l):
                        # t1tau[p=(k1,k2)] = ics[k2] - ics[k1-1]
                        seg = dl[:, :R, :, off[k1]:off[k1 + 1]]
                        if k1 == 0:
                            V.tensor_copy(seg, zc4[:, :, :, 0:L])
                        else:
                            V.tensor_tensor(
                                seg, zc4[:, :, :, k1:L],
                                zc4[:, :, :, k1 - 1:k1]
                                .broadcast_to([P, R, C, L - k1]),
                                op=OP.subtract)
                    V.scalar_tensor_tensor(dl[:, :R], sx[:, blo:bhi], tau_mu,
                                           dl[:, :R], op0=OP.mult,
                                           op1=OP.subtract)
                    # mu += delta ; m = (s - mu_new) - delta  (= s - mb)
                    V.tensor_tensor(mux[:, blo:bhi], mux[:, blo:bhi],
                                    dl[:, :R], op=OP.add)
                    V.tensor_tensor(sx[:, blo:bhi], sx[:, blo:bhi],
                                    mux[:, blo:bhi], op=OP.subtract)
                    V.tensor_tensor(sx[:, blo:bhi], sx[:, blo:bhi],
                                    dl[:, :R], op=OP.subtract)
                # l2proj: s *= nu / max(|m|, nu)
                n2 = btile("dl")
                tb = btile("tb")
                V.tensor_tensor(n2[:, :R], s1[:, blo:bhi], s1[:, blo:bhi],
                                op=OP.mult)
                V.tensor_tensor(tb[:, :R], s2[:, blo:bhi], s2[:, blo:bhi],
                                op=OP.mult)
                V.tensor_tensor(n2[:, :R], n2[:, :R], tb[:, :R], op=OP.add)
                S.activation(n2[:, :R], n2[:, :R], AF.Sqrt)
                V.tensor_scalar(n2[:, :R], n2[:, :R], 1.0 / nu, 1.0,
                                op0=OP.mult, op1=OP.max)
                V.reciprocal(n2[:, :R], n2[:, :R])
                V.tensor_tensor(s1[:, blo:bhi], s1[:, blo:bhi], n2[:, :R],
                                op=OP.mult)
                V.tensor_tensor(s2[:, blo:bhi], s2[:, blo:bhi], n2[:, :R],
                                op=OP.mult)

            # ======== C phase: clipping ========
            nc.sync.dma_start(wsp[1:P, lo:hi].unsqueeze(2),
                              p2[0:P - 1, lo:hi, C - 1:C])
            for (alo, ahi) in _blocks(lo, hi, AB):
                R = ahi - alo
                pa = atile("u1")
                pc = atile("u2")
                acc = atile("u3")
                dw = atile("q2")
                # d1 = p1[r]*A[r] - p1[r-1]*C[r-1]
                V.tensor_tensor(pa[:, :R], p1[:, alo:ahi],
                                bcast_h(mA, alo, ahi, L), op=OP.mult)
                V.tensor_tensor(pc[:, :R], p1[:, alo - 1:ahi - 1],
                                bcast_h(mC, alo - 1, ahi - 1, L), op=OP.mult)
                V.tensor_tensor(acc[:, :R], pa[:, :R], pc[:, :R],
                                op=OP.subtract)
                # d2 (w-adjoint): dw[w] = p2[w] - p2[w-1]
                # (wsp[0] == 0 gives the w=0 edge; wA kills p2[W-1] term)
                if C > 1:
                    if C > 2:
                        V.tensor_tensor(dw[:, :R, 1:C - 1],
                                        p2[:, alo:ahi, 1:C - 1],
                                        p2[:, alo:ahi, 0:C - 2], op=OP.subtract)
                    V.scalar_tensor_tensor(dw[:, :R, C - 1:C],
                                           p2[:, alo:ahi, C - 1:C],
                                           wm[:, 0:1],
                                           p2[:, alo:ahi, C - 2:C - 1],
                                           op0=OP.mult, op1=OP.subtract)
                    V.tensor_tensor(dw[:, :R, 0:1], p2[:, alo:ahi, 0:1],
                                    wsp[:, alo:ahi].unsqueeze(2),
                                    op=OP.subtract)
                else:
                    V.scalar_tensor_tensor(dw[:, :R, 0:1],
                                           p2[:, alo:ahi, 0:1], wm[:, 0:1],
                                           wsp[:, alo:ahi].unsqueeze(2),
                                           op0=OP.mult, op1=OP.subtract)
                V.tensor_tensor(acc[:, :R], acc[:, :R], dw[:, :R], op=OP.add)
                # d3 (z-adjoint)
                V.tensor_tensor(dw[:, :R, :, 1:L], p3[:, alo:ahi, :, 1:L],
                                p3[:, alo:ahi, :, 0:L - 1], op=OP.subtract)
                V.tensor_copy(dw[:, :R, :, 0:1], p3[:, alo:ahi, :, 0:1])
                # note: z=L-1 of dw would be -p3[L-2] + p3[L-1] from the sub;
                # true adjoint needs p3eff[L-1]=0 -> overwrite:
                V.tensor_scalar_mul(dw[:, :R, :, L - 1:L],
                                    p3[:, alo:ahi, :, L - 2:L - 1], -1.0)
                V.tensor_tensor(acc[:, :R], acc[:, :R], dw[:, :R], op=OP.add)
                # un = clip(u + tauu*acc); boundary z sets; ubar = 2un - u
                unw = atile("unw", F32)
                V.scalar_tensor_tensor(unw[:, :R], acc[:, :R], tauu,
                                       u[:, alo:ahi], op0=OP.mult, op1=OP.add)
                V.tensor_scalar(unw[:, :R], unw[:, :R], 0.0, 1.0,
                                op0=OP.max, op1=OP.min)
                V.memset(unw[:, :R, :, 0:1], 1.0)
                V.memset(unw[:, :R, :, L - 1:L], 0.0)
                V.scalar_tensor_tensor(ubar[:, alo:ahi], unw[:, :R], 2.0,
                                       u[:, alo:ahi], op0=OP.mult,
                                       op1=OP.subtract)
                V.tensor_copy(u[:, alo:ahi], unw[:, :R])

        # ---------------- output ----------------
        nc.sync.dma_start(u_out.ap(), flat(u[:, G:G + ROWS]))

    nc.compile()
    return nc


_cache = {}


def _get_program(lmbda, nu, repeats, l, cfg_key=None):
    key = (float(lmbda), float(nu), int(repeats), int(l))
    if key not in _cache:
        _cache[key] = build_program(float(lmbda), float(nu), int(repeats),
                                    int(l))
    return _cache[key]


def make_inputs(f, repeats, cfg=None):
    cfg = cfg or CFG
    H, W, L, NCORES, P = cfg["H"], cfg["W"], cfg["L"], cfg["NCORES"], cfg["P"]
    C = W // P
    ROWS = H // NCORES
    G = int(repeats)
    SLAB = ROWS + 2 * G
    f2 = np.asarray(f, dtype=np.float32).reshape(H, W)
    fpad = np.zeros((H + 2 * G, W), np.float32)
    fpad[G:G + H] = f2
    in_maps = []
    for k in range(NCORES):
        slab = fpad[k * ROWS: k * ROWS + SLAB]              # [SLAB, W]
        arr = slab.reshape(SLAB, P, C).transpose(1, 0, 2)   # [P, SLAB, C]
        g = np.arange(SLAB) + k * ROWS - G                  # global row ids
        mAv = ((g >= 0) & (g <= H - 2)).astype(np.float16)
        mCv = ((g >= 0) & (g <= H - 1)).astype(np.float16)
        wmv = np.ones((P, 2), np.float32)
        wmv[:, 1] = -1.0
        wmv[P - 1, :] = 0.0
        in_maps.append({
            "f_in": np.ascontiguousarray(arr.reshape(P, SLAB * C)),
            "mA_in": np.ascontiguousarray(np.broadcast_to(mAv, (P, SLAB))),
            "mC_in": np.ascontiguousarray(np.broadcast_to(mCv, (P, SLAB))),
            "wm_in": wmv,
        })
    return in_maps


def assemble_output(results, repeats, cfg=None):
    cfg = cfg or CFG
    H, W, L, NCORES, P = cfg["H"], cfg["W"], cfg["L"], cfg["NCORES"], cfg["P"]
    C = W // P
    ROWS = H // NCORES
    out = np.empty((H, W, 1, L), np.float32)
    for k in range(NCORES):
        o = results[k]["u_out"].reshape(P, ROWS, C, L)
        out[k * ROWS:(k + 1) * ROWS, :, 0, :] = (
            o.transpose(1, 0, 2, 3).reshape(ROWS, W, L))
    return out


def kernel(f, lmbda, nu, repeats, l):
    nc = _get_program(lmbda, nu, repeats, l)
    in_maps = make_inputs(f, repeats)
    res = run_bass_kernel_spmd(nc, in_maps, core_ids=list(range(CFG["NCORES"])))
    return assemble_output(res.results, repeats)


# revision 14
# speedup vs baseline: 1.1873x; 1.1873x over previous
"""Trainium2 Bass kernel for nn_PrimalDual (primal-dual multi-label segmentation).

Strategy:
  - Shard the image rows (h) across 8 cores; each core owns ROWS=48 output rows
    plus G=repeats ghost rows on each side computed redundantly, so no
    inter-core communication is needed (the ghost region shrinks by one row per
    iteration and is exactly exhausted after `repeats` iterations).
  - All state lives in SBUF for the whole solve: u (f32), ubar/p1/p2/p3 (f16),
    s1/s2/mu1/mu2 (f16, the proj=78-sized dual variables).
  - Layout: partition q in [0,128) holds image columns w = C*q + c, c in [0,C),
    C = W/128; free dims are (h_local, c, z|proj).
  - The einsum mu->z and the interval sums z->proj are done with segmented
    scans (tensor_tensor_scan) plus grouped strided subtract ops; everything
    else is pointwise chains on DVE/ACT.
"""

import numpy as np
from contextlib import ExitStack

import concourse.bass as bass
import concourse.tile as tile
from concourse import bacc, mybir
from concourse.bass_utils import run_bass_kernel_spmd

F16 = mybir.dt.float16
U8 = mybir.dt.uint8
F32 = mybir.dt.float32
AF = mybir.ActivationFunctionType
OP = mybir.AluOpType

# problem geometry (from spec; patchable for small-config sim tests)
CFG = dict(H=384, W=384, L=12, NCORES=8, P=128)

AB = 8    # A/C-phase row-block
BB = 10   # B-phase row-block
MB = 20   # mu-sum scan/fold row-block

_HALF_PI = 1.5707963267948966


def flat(ap):
    nd = len(ap.shape)
    if nd == 2:
        return ap
    names = " ".join(f"d{i}" for i in range(nd - 1))
    return ap.rearrange(f"p {names} -> p ({names})")


def _register_consts(nc, values):
    for v in values:
        v = float(v)
        if (mybir.dt.float32, v) in nc.const_aps.aps:
            continue
        t = nc.alloc_sbuf_tensor(f"constf32-{len(nc.const_aps.aps)}", [128, 1], F32)
        nc.gpsimd.memset(t.ap(), v)
        nc.const_aps.aps[(mybir.dt.float32, v)] = t.ap()
    nc.all_engine_barrier()


def _blocks(lo, hi, step):
    out = []
    r = lo
    while r < hi:
        out.append((r, min(r + step, hi)))
        r = out[-1][1]
    return out


def build_program(lmbda, nu, repeats, l, cfg=None):
    cfg = cfg or CFG
    H, W, L, NCORES, P = cfg["H"], cfg["W"], cfg["L"], cfg["NCORES"], cfg["P"]
    assert L == l
    assert W % P == 0
    C = W // P
    ROWS = H // NCORES
    G = repeats
    SLAB = ROWS + 2 * G
    PROJ = l * (l + 1) // 2

    sigmap = 1.0 / (3.0 + l)
    tauu = 1.0 / 6.0
    tau_mu = 1.0 / (2.0 + PROJ / 4.0)
    lmbda = float(lmbda)
    nu = float(nu)
    sql = float(np.sqrt(lmbda))
    kl = [(z + 1) / l for z in range(l)]

    # run offsets: off(k1) = start index of the k1-run in p-order (k1-major)
    off = [0] * (l + 1)
    for k1 in range(l):
        off[k1 + 1] = off[k1] + (l - k1)

    nc = bacc.Bacc("TRN2", target_bir_lowering=False, debug=False,
                   num_devices=NCORES)
    _register_consts(nc, [sql * k for k in kl] + [2.0 / 3.0, _HALF_PI])

    f_in = nc.dram_tensor("f_in", [P, SLAB * C], F32, kind="ExternalInput")
    mA_in = nc.dram_tensor("mA_in", [P, SLAB], F16, kind="ExternalInput")
    mC_in = nc.dram_tensor("mC_in", [P, SLAB], F16, kind="ExternalInput")
    wm_in = nc.dram_tensor("wm_in", [P, 2], F32, kind="ExternalInput")
    u_out = nc.dram_tensor("u_out", [P, ROWS * C * L], F32, kind="ExternalOutput")

    with tile.TileContext(nc) as tc, ExitStack() as ctx, \
            nc.allow_low_precision(reason="f16 state by design"):
        V = nc.vector
        S = nc.scalar

        st = ctx.enter_context(tc.tile_pool(name="state", bufs=1))
        u = st.tile([P, SLAB, C, L], F32)
        ubar = st.tile([P, SLAB, C, L], F16)
        p1 = st.tile([P, SLAB, C, L], F16)
        p2 = st.tile([P, SLAB, C, L], F16)
        p3 = st.tile([P, SLAB, C, L], F16)
        s1 = st.tile([P, SLAB, C, PROJ], F16)
        s2 = st.tile([P, SLAB, C, PROJ], F16)
        mu1 = st.tile([P, SLAB, C, PROJ], F16)
        mu2 = st.tile([P, SLAB, C, PROJ], F16)
        ld2 = st.tile([P, SLAB, C, L], F16)
        fsb = st.tile([P, SLAB, C], F32)
        mA = st.tile([P, SLAB], F16)
        mC = st.tile([P, SLAB], F16)
        zmb = st.tile([P, BB, C, L], F16)     # z-segment mask block (0 at z=0)
        zmbF = st.tile([P, MB, C, L], F16)    # z-segment mask (msum scan)
        pmb = st.tile([P, MB, C, PROJ], F16)  # proj-segment mask block
        msum1 = st.tile([P, SLAB, C, L], F16)  # mu1 -> z sums (unscaled)
        msum2 = st.tile([P, SLAB, C, L], F16)
        # w-shift staging (cross-partition neighbours via DMA)
        wm = st.tile([P, 2], F32)             # [wA, -wA] per-partition
        wsu = st.tile([P, SLAB, L], F16)      # ubar[q+1, c=0] staged at q
        wsp = st.tile([P, SLAB, L], F16)      # p2[q-1, c=C-1] staged at q

        at_ = ctx.enter_context(tc.tile_pool(name="atemp", bufs=2))
        bt_ = ctx.enter_context(tc.tile_pool(name="btemp", bufs=1))
        ct_ = ctx.enter_context(tc.tile_pool(name="csppool", bufs=1))

        def atile(tag, dt=F16):
            return at_.tile([P, AB, C, L], dt, tag=tag, name=tag)

        def btile(tag, dt=F16):
            return bt_.tile([P, BB, C, PROJ], dt, tag=tag, name=tag)

        def bcast_h(m, lo, hi, last):
            return m[:, lo:hi].unsqueeze(2).unsqueeze(3).broadcast_to(
                [P, hi - lo, C, last])

        # ---------------- init ----------------
        nc.sync.dma_start(flat(fsb[:]), f_in.ap())
        nc.sync.dma_start(mA[:], mA_in.ap())
        nc.sync.dma_start(mC[:], mC_in.ap())
        nc.sync.dma_start(wm[:], wm_in.ap())
        fb = fsb[:].unsqueeze(3).broadcast_to([P, SLAB, C, L])
        V.tensor_copy(u[:], fb)
        V.tensor_copy(ubar[:], fb)
        for z in range(L):
            S.activation(ld2[:, :, :, z:z + 1], fsb[:].unsqueeze(3),
                         AF.Square, scale=-sql, bias=sql * kl[z])
        for t in (p1, p2, p3, s1, s2, mu1, mu2):
            nc.gpsimd.memset(t[:], 0.0)
        V.memset(zmb[:], 1.0)
        V.memset(zmb[:, :, :, 0:1], 0.0)
        V.memset(zmbF[:], 1.0)
        V.memset(zmbF[:, :, :, 0:1], 0.0)
        V.memset(pmb[:], 1.0)
        V.memset(pmb[:, :, :, 0:1], 0.0)
        V.memset(wsu[:], 0.0)
        V.memset(wsp[:], 0.0)

        # ---------------- iterations ----------------
        for it in range(repeats):
            lo, hi = it + 1, SLAB - 1 - it
            if NCORES == 1:
                lo, hi = G, G + ROWS  # no ghost shrink needed, masks do edges
            # A/B phases need one extra row above: clipping at row r consumes
            # the same-iteration parabola output at r-1.
            ablo = max(lo - 1, 0)

            # stage w-neighbours for the whole row range
            nc.sync.dma_start(wsu[0:P - 1, ablo:hi].unsqueeze(2),
                              ubar[1:P, ablo:hi, 0:1])
            # ubar[w+1] for w=W-1 is "replicate last": A-mask kills u2 there,
            # but keep the stale zeros in wsu row P-1 (never read: see memset).

            # ======== mu -> z sums (msum1/msum2, unscaled) ========
            for (mlo, mhi) in _blocks(ablo, hi, MB):
                RW = mhi - mlo
                for (mus, msum) in ((mu1, msum1), (mu2, msum2)):
                    csp = ct_.tile([P, MB * C * PROJ], F16, tag="csp",
                                   name="csp")
                    V.tensor_tensor_scan(
                        csp[:, :RW * C * PROJ], flat(pmb[:, :RW]),
                        flat(mus[:, mlo:mhi]), 0.0, op0=OP.mult, op1=OP.add)
                    cs4 = csp[:, :RW * C * PROJ].rearrange(
                        "p (r c j) -> p r c j", r=RW, c=C, j=PROJ)
                    ms = msum[:, mlo:mhi]
                    # msum[z] = sum_{k1<=z} cs[off(k1+1)-1] - cs[off(k1)+z-k1-1]
                    # F part: gather run-total cumulatives T[k1], then a
                    # segmented cumsum over k1 directly into msum.
                    tg = ct_.tile([P, MB, C, L], F16, tag="tg", name="tg")
                    for k1 in range(l):
                        V.tensor_scalar_mul(
                            tg[:, :RW, :, k1:k1 + 1],
                            cs4[:, :, :, off[k1 + 1] - 1:off[k1 + 1]], 1.0)
                    V.tensor_tensor_scan(
                        flat(ms), flat(zmb[:, :1]).broadcast_to(
                            [P, RW * C * L]) if False else flat(zmbF[:, :RW]),
                        flat(tg[:, :RW]), 0.0, op0=OP.mult, op1=OP.add)
                    for k1 in range(l):
                        z0 = max(k1, 1)
                        a = off[k1] + z0 - k1 - 1
                        V.tensor_tensor(ms[:, :, :, z0:L], ms[:, :, :, z0:L],
                                        cs4[:, :, :, a:a + (L - z0)],
                                        op=OP.subtract)

            # ======== A phase: parabola ========
            for (alo, ahi) in _blocks(ablo, hi, AB):
                R = ahi - alo

                def asl(tl, s=0, e=None):
                    return tl[:, alo + s: ahi + (e or 0)]

                u1 = atile("u1")
                u2 = atile("u2")
                u3 = atile("u3")
                # u1 = (ubar[r+1]-ubar[r]) * A
                V.tensor_tensor(u1[:, :R], ubar[:, alo + 1:ahi + 1],
                                ubar[:, alo:ahi], op=OP.subtract)
                V.tensor_tensor(u1[:, :R], u1[:, :R],
                                bcast_h(mA, alo, ahi, L), op=OP.mult)
                V.tensor_tensor(u1[:, :R], u1[:, :R], msum1[:, alo:ahi],
                                op=OP.add)
                V.scalar_tensor_tensor(u1[:, :R], u1[:, :R], sigmap,
                                       p1[:, alo:ahi], op0=OP.mult, op1=OP.add)
                # u2 = (ubar[w+1]-ubar[w]); w=W-1 -> 0
                if C > 1:
                    V.tensor_tensor(u2[:, :R, 0:C - 1],
                                    ubar[:, alo:ahi, 1:C],
                                    ubar[:, alo:ahi, 0:C - 1], op=OP.subtract)
                V.scalar_tensor_tensor(u2[:, :R, C - 1:C],
                                       ubar[:, alo:ahi, C - 1:C],
                                       wm[:, 1:2], wsu[:, alo:ahi].unsqueeze(2),
                                       op0=OP.mult, op1=OP.add)
                V.tensor_tensor(u2[:, :R], u2[:, :R], msum2[:, alo:ahi],
                                op=OP.add)
                V.scalar_tensor_tensor(u2[:, :R], u2[:, :R], sigmap,
                                       p2[:, alo:ahi], op0=OP.mult, op1=OP.add)
                # u3 = dz(ubar); z=L-1 -> 0
                V.tensor_tensor(u3[:, :R, :, 0:L - 1],
                                ubar[:, alo:ahi, :, 1:L],
                                ubar[:, alo:ahi, :, 0:L - 1], op=OP.subtract)
                V.memset(u3[:, :R, :, L - 1:L], 0.0)
                V.scalar_tensor_tensor(u3[:, :R], u3[:, :R], sigmap,
                                       p3[:, alo:ahi], op0=OP.mult, op1=OP.add)

                # cubic solve
                q2 = atile("q2")
                tq = atile("tq")
                S.activation(q2[:, :R], u1[:, :R], AF.Square)
                S.activation(tq[:, :R], u2[:, :R], AF.Square)
                V.tensor_tensor(q2[:, :R], q2[:, :R], tq[:, :R], op=OP.add)
                bv = atile("dd")
                V.scalar_tensor_tensor(bv[:, :R], q2[:, :R], 0.25,
                                       ld2[:, alo:ahi], op0=OP.mult,
                                       op1=OP.subtract)
                msk = atile("msk", U8)
                V.tensor_tensor(msk[:, :R], u3[:, :R], bv[:, :R], op=OP.is_lt)
                bq = atile("bq")
                V.tensor_tensor(bq[:, :R], u3[:, :R], ld2[:, alo:ahi], op=OP.add)
                S.activation(bq[:, :R], bq[:, :R], AF.Identity,
                             scale=-1.0 / 3.0, bias=2.0 / 3.0)
                b3 = atile("b3")
                V.tensor_tensor(b3[:, :R], bq[:, :R], bq[:, :R], op=OP.mult)
                V.tensor_tensor(b3[:, :R], b3[:, :R], bq[:, :R], op=OP.mult)
                dd = atile("dd")
                V.scalar_tensor_tensor(dd[:, :R], q2[:, :R], 0.25, b3[:, :R],
                                       op0=OP.mult, op1=OP.add)
                dneg = atile("dneg", U8)
                V.tensor_scalar(dneg[:, :R], dd[:, :R], 0.0, None, op0=OP.is_lt)
                norm = atile("norm")
                S.activation(norm[:, :R], q2[:, :R], AF.Sqrt)
                # c = cbrt(0.5*norm + sqrt(max(d,0)))
                sq = atile("sq")
                S.activation(sq[:, :R], dd[:, :R], AF.Relu)
                S.activation(sq[:, :R], sq[:, :R], AF.Sqrt)
                V.scalar_tensor_tensor(sq[:, :R], norm[:, :R], 0.5, sq[:, :R],
                                       op0=OP.mult, op1=OP.add)
                cc = atile("cc")
                S.activation(cc[:, :R], sq[:, :R], AF.Ln)
                S.activation(cc[:, :R], cc[:, :R], AF.Exp, scale=1.0 / 3.0)
                rc = atile("rc")
                V.reciprocal(rc[:, :R], cc[:, :R])
                vv = atile("vv")
                V.tensor_tensor(vv[:, :R], bq[:, :R], rc[:, :R], op=OP.mult)
                V.tensor_tensor(vv[:, :R], cc[:, :R], vv[:, :R], op=OP.subtract)
                # trig branch: v = 2*sb*cos(arccos(ratio)/3), ratio=norm/(2*sb3)
                sb3 = atile("sb3")
                S.activation(sb3[:, :R], b3[:, :R], AF.Relu, scale=-1.0)
                S.activation(sb3[:, :R], sb3[:, :R], AF.Sqrt, scale=4.0)
                V.reciprocal(sb3[:, :R], sb3[:, :R])
                rat = atile("sq")
                V.tensor_tensor(rat[:, :R], norm[:, :R], sb3[:, :R], op=OP.mult)
                V.tensor_scalar(rat[:, :R], rat[:, :R], 0.0, 1.0,
                                op0=OP.max, op1=OP.min)
                # t = sqrt((1-r)/(1+r)); theta = 2*atan(t)
                den = atile("dd")
                V.tensor_scalar(den[:, :R], rat[:, :R], 1.0, None, op0=OP.add)
                V.reciprocal(den[:, :R], den[:, :R])
                V.tensor_scalar(rat[:, :R], rat[:, :R], -1.0, 1.0,
                                op0=OP.mult, op1=OP.add)
                V.tensor_tensor(rat[:, :R], rat[:, :R], den[:, :R], op=OP.mult)
                S.activation(rat[:, :R], rat[:, :R], AF.Sqrt)
                S.activation(rat[:, :R], rat[:, :R], AF.Arctan)
                # v_s = sin(pi/2 - (2/3)atan) = cos(theta/3)
                S.activation(rat[:, :R], rat[:, :R], AF.Sin,
                             scale=-2.0 / 3.0, bias=_HALF_PI)
                sb2 = atile("b3")
                S.activation(sb2[:, :R], bq[:, :R], AF.Relu, scale=-1.0)
                S.activation(sb2[:, :R], sb2[:, :R], AF.Sqrt, scale=4.0)
                V.tensor_tensor(sb2[:, :R], sb2[:, :R], rat[:, :R], op=OP.mult)
                V.copy_predicated(vv[:, :R], dneg[:, :R], sb2[:, :R])
                # scale = 2*v/norm, guarded by norm>0
                V.reciprocal(norm[:, :R], norm[:, :R])
                V.scalar_tensor_tensor(vv[:, :R], vv[:, :R], 2.0, norm[:, :R],
                                       op0=OP.mult, op1=OP.mult)
                nzm = atile("nzm", U8)
                V.tensor_scalar(nzm[:, :R], q2[:, :R], 0.0, None, op0=OP.is_gt)
                V.tensor_tensor(nzm[:, :R], nzm[:, :R], msk[:, :R],
                                op=OP.logical_and)
                # p1,p2 update (in place)
                gu = atile("cc")
                V.tensor_tensor(gu[:, :R], vv[:, :R], u1[:, :R], op=OP.mult)
                S.activation(p1[:, alo:ahi], u1[:, :R], AF.Copy)
                V.copy_predicated(p1[:, alo:ahi], nzm[:, :R], gu[:, :R])
                V.tensor_tensor(gu[:, :R], vv[:, :R], u2[:, :R], op=OP.mult)
                S.activation(p2[:, alo:ahi], u2[:, :R], AF.Copy)
                V.copy_predicated(p2[:, alo:ahi], nzm[:, :R], gu[:, :R])
                # p3 = where(mask, 0.25*(p1n^2+p2n^2) - ld2, u3)
                tq2 = atile("tq")
                S.activation(q2[:, :R], p1[:, alo:ahi], AF.Square)
                S.activation(tq2[:, :R], p2[:, alo:ahi], AF.Square)
                V.tensor_tensor(q2[:, :R], q2[:, :R], tq2[:, :R], op=OP.add)
                V.scalar_tensor_tensor(q2[:, :R], q2[:, :R], 0.25,
                                       ld2[:, alo:ahi], op0=OP.mult,
                                       op1=OP.subtract)
                S.activation(p3[:, alo:ahi], u3[:, :R], AF.Copy)
                V.copy_predicated(p3[:, alo:ahi], msk[:, :R], q2[:, :R])

            # ======== B phase: interval sums, mu update, l2proj ========
            # mu/s are only consumed by the next iteration's A phase, whose
            # row range is [lo, hi-1).
            bhi_all = hi - 1 if NCORES > 1 else hi
            for (blo, bhi) in _blocks(lo, bhi_all, BB):
                R = bhi - blo
                zc1 = bt_.tile([P, BB * C * L], F16, tag="zc1", name="zc1")
                zc2 = bt_.tile([P, BB * C * L], F16, tag="zc2", name="zc2")
                pt = bt_.tile([P, BB, C, L], F16, tag="pt", name="pt")
                for (pn, zc) in ((p1, zc1), (p2, zc2)):
                    V.tensor_scalar_mul(pt[:, :R], pn[:, blo:bhi], tau_mu)
                    V.tensor_tensor_scan(
                        zc[:, :R * C * L], flat(zmb[:, :R]), flat(pt[:, :R]),
                        0.0, op0=OP.mult, op1=OP.add)
                for (sx, mux, zc) in ((s1, mu1, zc1), (s2, mu2, zc2)):
                    zc4 = zc[:, :R * C * L].rearrange(
                        "p (r c z) -> p r c z", r=R, c=C, z=L)
                    # delta = tau*(s - t1): build t1tau into dl then finish
                    dl = btile("dl")
                    for k1 in range(l):
                        # t1tau[p=(k1,k2)] = ics[k2] - ics[k1-1]
                        seg = dl[:, :R, :, off[k1]:off[k1 + 1]]
                        if k1 == 0:
                            S.activation(seg, zc4[:, :, :, 0:L], AF.Copy)
                        else:
                            V.tensor_tensor(
                                seg, zc4[:, :, :, k1:L],
                                zc4[:, :, :, k1 - 1:k1]
                                .broadcast_to([P, R, C, L - k1]),
                                op=OP.subtract)
                    ts_ = btile("tb")
                    V.tensor_scalar_mul(ts_[:, :R], sx[:, blo:bhi], tau_mu)
                    V.tensor_tensor(dl[:, :R], ts_[:, :R], dl[:, :R],
                                    op=OP.subtract)
                    # mu += delta ; m = (s - mu_new) - delta  (= s - mb)
                    V.tensor_tensor(mux[:, blo:bhi], mux[:, blo:bhi],
                                    dl[:, :R], op=OP.add)
                    V.tensor_tensor(sx[:, blo:bhi], sx[:, blo:bhi],
                                    mux[:, blo:bhi], op=OP.subtract)
                    V.tensor_tensor(sx[:, blo:bhi], sx[:, blo:bhi],
                                    dl[:, :R], op=OP.subtract)
                # l2proj: s *= nu / max(|m|, nu)
                n2 = btile("dl")
                tb = btile("tb")
                S.activation(n2[:, :R], s1[:, blo:bhi], AF.Square)
                S.activation(tb[:, :R], s2[:, blo:bhi], AF.Square)
                V.tensor_tensor(n2[:, :R], n2[:, :R], tb[:, :R], op=OP.add)
                S.activation(n2[:, :R], n2[:, :R], AF.Sqrt)
                V.tensor_scalar(n2[:, :R], n2[:, :R], 1.0 / nu, 1.0,
                                op0=OP.mult, op1=OP.max)
                V.reciprocal(n2[:, :R], n2[:, :R])
                V.tensor_tensor(s1[:, blo:bhi], s1[:, blo:bhi], n2[:, :R],
                                op=OP.mult)
                V.tensor_tensor(s2[:, blo:bhi], s2[:, blo:bhi], n2[:, :R],
                                op=OP.mult)

            # ======== C phase: clipping ========
            nc.sync.dma_start(wsp[1:P, lo:hi].unsqueeze(2),
                              p2[0:P - 1, lo:hi, C - 1:C])
            for (alo, ahi) in _blocks(lo, hi, AB):
                R = ahi - alo
                pa = atile("u1")
                pc = atile("u2")
                acc = atile("u3")
                dw = atile("q2")
                # d1 = p1[r]*A[r] - p1[r-1]*C[r-1]
                V.tensor_tensor(pa[:, :R], p1[:, alo:ahi],
                                bcast_h(mA, alo, ahi, L), op=OP.mult)
                V.tensor_tensor(pc[:, :R], p1[:, alo - 1:ahi - 1],
                                bcast_h(mC, alo - 1, ahi - 1, L), op=OP.mult)
                V.tensor_tensor(acc[:, :R], pa[:, :R], pc[:, :R],
                                op=OP.subtract)
                # d2 (w-adjoint): dw[w] = p2[w] - p2[w-1]
                # (wsp[0] == 0 gives the w=0 edge; wA kills p2[W-1] term)
                if C > 1:
                    if C > 2:
                        V.tensor_tensor(dw[:, :R, 1:C - 1],
                                        p2[:, alo:ahi, 1:C - 1],
                                        p2[:, alo:ahi, 0:C - 2], op=OP.subtract)
                    V.scalar_tensor_tensor(dw[:, :R, C - 1:C],
                                           p2[:, alo:ahi, C - 1:C],
                                           wm[:, 0:1],
                                           p2[:, alo:ahi, C - 2:C - 1],
                                           op0=OP.mult, op1=OP.subtract)
                    V.tensor_tensor(dw[:, :R, 0:1], p2[:, alo:ahi, 0:1],
                                    wsp[:, alo:ahi].unsqueeze(2),
                                    op=OP.subtract)
                else:
                    V.scalar_tensor_tensor(dw[:, :R, 0:1],
                                           p2[:, alo:ahi, 0:1], wm[:, 0:1],
                                           wsp[:, alo:ahi].unsqueeze(2),
                                           op0=OP.mult, op1=OP.subtract)
                V.tensor_tensor(acc[:, :R], acc[:, :R], dw[:, :R], op=OP.add)
                # d3 (z-adjoint)
                V.tensor_tensor(dw[:, :R, :, 1:L], p3[:, alo:ahi, :, 1:L],
                                p3[:, alo:ahi, :, 0:L - 1], op=OP.subtract)
                V.tensor_copy(dw[:, :R, :, 0:1], p3[:, alo:ahi, :, 0:1])
                # note: z=L-1 of dw would be -p3[L-2] + p3[L-1] from the sub;
                # true adjoint needs p3eff[L-1]=0 -> overwrite:
                V.tensor_scalar_mul(dw[:, :R, :, L - 1:L],
                                    p3[:, alo:ahi, :, L - 2:L - 1], -1.0)
                V.tensor_tensor(acc[:, :R], acc[:, :R], dw[:, :R], op=OP.add)
                # un = clip(u + tauu*acc); boundary z sets; ubar = 2un - u
                unw = atile("unw", F32)
                V.scalar_tensor_tensor(unw[:, :R], acc[:, :R], tauu,
                                       u[:, alo:ahi], op0=OP.mult, op1=OP.add)
                V.tensor_scalar(unw[:, :R], unw[:, :R], 0.0, 1.0,
                                op0=OP.max, op1=OP.min)
                V.memset(unw[:, :R, :, 0:1], 1.0)
                V.memset(unw[:, :R, :, L - 1:L], 0.0)
                V.scalar_tensor_tensor(ubar[:, alo:ahi], unw[:, :R], 2.0,
                                       u[:, alo:ahi], op0=OP.mult,
                                       op1=OP.subtract)
                S.activation(u[:, alo:ahi], unw[:, :R], AF.Copy)

        # ---------------- output ----------------
        nc.sync.dma_start(u_out.ap(), flat(u[:, G:G + ROWS]))

    nc.compile()
    return nc


_cache = {}


def _get_program(lmbda, nu, repeats, l, cfg_key=None):
    key = (float(lmbda), float(nu), int(repeats), int(l))
    if key not in _cache:
        _cache[key] = build_program(float(lmbda), float(nu), int(repeats),
                                    int(l))
    return _cache[key]


def make_inputs(f, repeats, cfg=None):
    cfg = cfg or CFG
    H, W, L, NCORES, P = cfg["H"], cfg["W"], cfg["L"], cfg["NCORES"], cfg["P"]
    C = W // P
    ROWS = H // NCORES
    G = int(repeats)
    SLAB = ROWS + 2 * G
    f2 = np.asarray(f, dtype=np.float32).reshape(H, W)
    fpad = np.zeros((H + 2 * G, W), np.float32)
    fpad[G:G + H] = f2
    in_maps = []
    for k in range(NCORES):
        slab = fpad[k * ROWS: k * ROWS + SLAB]              # [SLAB, W]
        arr = slab.reshape(SLAB, P, C).transpose(1, 0, 2)   # [P, SLAB, C]
        g = np.arange(SLAB) + k * ROWS - G                  # global row ids
        mAv = ((g >= 0) & (g <= H - 2)).astype(np.float16)
        mCv = ((g >= 0) & (g <= H - 1)).astype(np.float16)
        wmv = np.ones((P, 2), np.float32)
        wmv[:, 1] = -1.0
        wmv[P - 1, :] = 0.0
        in_maps.append({
            "f_in": np.ascontiguousarray(arr.reshape(P, SLAB * C)),
            "mA_in": np.ascontiguousarray(np.broadcast_to(mAv, (P, SLAB))),
            "mC_in": np.ascontiguousarray(np.broadcast_to(mCv, (P, SLAB))),
            "wm_in": wmv,
        })
    return in_maps


def assemble_output(results, repeats, cfg=None):
    cfg = cfg or CFG
    H, W, L, NCORES, P = cfg["H"], cfg["W"], cfg["L"], cfg["NCORES"], cfg["P"]
    C = W // P
    ROWS = H // NCORES
    out = np.empty((H, W, 1, L), np.float32)
    for k in range(NCORES):
        o = results[k]["u_out"].reshape(P, ROWS, C, L)
        out[k * ROWS:(k + 1) * ROWS, :, 0, :] = (
            o.transpose(1, 0, 2, 3).reshape(ROWS, W, L))
    return out


def kernel(f, lmbda, nu, repeats, l):
    l = int(l)
    repeats = int(repeats)
    cfg = dict(CFG)
    cfg["L"] = l
    key = (float(lmbda), float(nu), repeats, l)
    if key not in _cache:
        _cache[key] = build_program(float(lmbda), float(nu), repeats, l,
                                    cfg=cfg)
    nc = _cache[key]
    in_maps = make_inputs(np.asarray(f, np.float32), repeats, cfg=cfg)
    res = run_bass_kernel_spmd(nc, in_maps,
                               core_ids=list(range(cfg["NCORES"])))
    return assemble_output(res.results, repeats, cfg=cfg)


# revision 15
# speedup vs baseline: 425.3293x; 358.2197x over previous
"""Trainium2 Bass kernel for nn_PrimalDual (primal-dual multi-label segmentation).

Strategy:
  - Shard the image rows (h) across 8 cores; each core owns ROWS=48 output rows
    plus G=repeats ghost rows on each side computed redundantly, so no
    inter-core communication is needed (the ghost region shrinks by one row per
    iteration and is exactly exhausted after `repeats` iterations).
  - All state lives in SBUF for the whole solve: u (f32), ubar/p1/p2/p3 (f16),
    s1/s2/mu1/mu2 (f16, the proj=78-sized dual variables).
  - Layout: partition q in [0,128) holds image columns w = C*q + c, c in [0,C),
    C = W/128; free dims are (h_local, c, z|proj).
  - The einsum mu->z and the interval sums z->proj are done with segmented
    scans (tensor_tensor_scan) plus grouped strided subtract ops; everything
    else is pointwise chains on DVE/ACT.
"""

import numpy as np
from contextlib import ExitStack

import concourse.bass as bass
import concourse.tile as tile
from concourse import bacc, mybir
from concourse.bass_utils import run_bass_kernel_spmd

F16 = mybir.dt.float16
U8 = mybir.dt.uint8
F32 = mybir.dt.float32
AF = mybir.ActivationFunctionType
OP = mybir.AluOpType

# problem geometry (from spec; patchable for small-config sim tests)
CFG = dict(H=384, W=384, L=12, NCORES=8, P=128)

AB = 8    # A/C-phase row-block
BB = 10   # B-phase row-block
MB = 20   # mu-sum scan/fold row-block

_HALF_PI = 1.5707963267948966


def flat(ap):
    nd = len(ap.shape)
    if nd == 2:
        return ap
    names = " ".join(f"d{i}" for i in range(nd - 1))
    return ap.rearrange(f"p {names} -> p ({names})")


def _register_consts(nc, values):
    for v in values:
        v = float(v)
        if (mybir.dt.float32, v) in nc.const_aps.aps:
            continue
        t = nc.alloc_sbuf_tensor(f"constf32-{len(nc.const_aps.aps)}", [128, 1], F32)
        nc.gpsimd.memset(t.ap(), v)
        nc.const_aps.aps[(mybir.dt.float32, v)] = t.ap()
    nc.all_engine_barrier()


def _blocks(lo, hi, step):
    out = []
    r = lo
    while r < hi:
        out.append((r, min(r + step, hi)))
        r = out[-1][1]
    return out


def build_program(lmbda, nu, repeats, l, cfg=None):
    cfg = cfg or CFG
    H, W, L, NCORES, P = cfg["H"], cfg["W"], cfg["L"], cfg["NCORES"], cfg["P"]
    assert L == l
    assert W % P == 0
    C = W // P
    ROWS = H // NCORES
    G = repeats
    SLAB = ROWS + 2 * G
    PROJ = l * (l + 1) // 2

    sigmap = 1.0 / (3.0 + l)
    tauu = 1.0 / 6.0
    tau_mu = 1.0 / (2.0 + PROJ / 4.0)
    lmbda = float(lmbda)
    nu = float(nu)
    sql = float(np.sqrt(lmbda))
    kl = [(z + 1) / l for z in range(l)]

    # run offsets: off(k1) = start index of the k1-run in p-order (k1-major)
    off = [0] * (l + 1)
    for k1 in range(l):
        off[k1 + 1] = off[k1] + (l - k1)

    nc = bacc.Bacc("TRN2", target_bir_lowering=False, debug=False,
                   num_devices=NCORES)
    _register_consts(nc, [sql * k for k in kl] + [2.0 / 3.0, _HALF_PI])

    f_in = nc.dram_tensor("f_in", [P, SLAB * C], F32, kind="ExternalInput")
    mA_in = nc.dram_tensor("mA_in", [P, SLAB], F16, kind="ExternalInput")
    mC_in = nc.dram_tensor("mC_in", [P, SLAB], F16, kind="ExternalInput")
    wm_in = nc.dram_tensor("wm_in", [P, 2], F32, kind="ExternalInput")
    u_out = nc.dram_tensor("u_out", [P, ROWS * C * L], F32, kind="ExternalOutput")

    with tile.TileContext(nc) as tc, ExitStack() as ctx, \
            nc.allow_low_precision(reason="f16 state by design"):
        V = nc.vector
        S = nc.scalar

        st = ctx.enter_context(tc.tile_pool(name="state", bufs=1))
        u = st.tile([P, SLAB, C, L], F32)
        ubar = st.tile([P, SLAB, C, L], F16)
        p1 = st.tile([P, SLAB, C, L], F16)
        p2 = st.tile([P, SLAB, C, L], F16)
        p3 = st.tile([P, SLAB, C, L], F16)
        s1 = st.tile([P, SLAB, C, PROJ], F16)
        s2 = st.tile([P, SLAB, C, PROJ], F16)
        mu1 = st.tile([P, SLAB, C, PROJ], F16)
        mu2 = st.tile([P, SLAB, C, PROJ], F16)
        ld2 = st.tile([P, SLAB, C, L], F16)
        fsb = st.tile([P, SLAB, C], F32)
        mA = st.tile([P, SLAB], F16)
        mC = st.tile([P, SLAB], F16)
        zmb = st.tile([P, BB, C, L], F16)     # z-segment mask block (0 at z=0)
        zmbF = st.tile([P, MB, C, L], F16)    # z-segment mask (msum scan)
        pmb = st.tile([P, MB, C, PROJ], F16)  # proj-segment mask block
        msum1 = st.tile([P, SLAB, C, L], F16)  # mu1 -> z sums (unscaled)
        msum2 = st.tile([P, SLAB, C, L], F16)
        # w-shift staging (cross-partition neighbours via DMA)
        wm = st.tile([P, 2], F32)             # [wA, -wA] per-partition
        wsu = st.tile([P, SLAB, L], F16)      # ubar[q+1, c=0] staged at q
        wsp = st.tile([P, SLAB, L], F16)      # p2[q-1, c=C-1] staged at q

        at_ = ctx.enter_context(tc.tile_pool(name="atemp", bufs=2))
        bt_ = ctx.enter_context(tc.tile_pool(name="btemp", bufs=1))
        ct_ = ctx.enter_context(tc.tile_pool(name="csppool", bufs=1))

        def atile(tag, dt=F16):
            return at_.tile([P, AB, C, L], dt, tag=tag, name=tag)

        def btile(tag, dt=F16):
            return bt_.tile([P, BB, C, PROJ], dt, tag=tag, name=tag)

        def bcast_h(m, lo, hi, last):
            return m[:, lo:hi].unsqueeze(2).unsqueeze(3).broadcast_to(
                [P, hi - lo, C, last])

        # ---------------- init ----------------
        nc.sync.dma_start(flat(fsb[:]), f_in.ap())
        nc.sync.dma_start(mA[:], mA_in.ap())
        nc.sync.dma_start(mC[:], mC_in.ap())
        nc.sync.dma_start(wm[:], wm_in.ap())
        fb = fsb[:].unsqueeze(3).broadcast_to([P, SLAB, C, L])
        V.tensor_copy(u[:], fb)
        V.tensor_copy(ubar[:], fb)
        for z in range(L):
            S.activation(ld2[:, :, :, z:z + 1], fsb[:].unsqueeze(3),
                         AF.Square, scale=-sql, bias=sql * kl[z])
        for t in (p1, p2, p3, s1, s2, mu1, mu2):
            nc.gpsimd.memset(t[:], 0.0)
        V.memset(zmb[:], 1.0)
        V.memset(zmb[:, :, :, 0:1], 0.0)
        V.memset(zmbF[:], 1.0)
        V.memset(zmbF[:, :, :, 0:1], 0.0)
        V.memset(pmb[:], 1.0)
        V.memset(pmb[:, :, :, 0:1], 0.0)
        V.memset(wsu[:], 0.0)
        V.memset(wsp[:], 0.0)

        # ---------------- iterations ----------------
        for it in range(repeats):
            lo, hi = it + 1, SLAB - 1 - it
            if NCORES == 1:
                lo, hi = G, G + ROWS  # no ghost shrink needed, masks do edges
            # A/B phases need one extra row above: clipping at row r consumes
            # the same-iteration parabola output at r-1.
            ablo = max(lo - 1, 0)

            # stage w-neighbours for the whole row range
            nc.sync.dma_start(wsu[0:P - 1, ablo:hi].unsqueeze(2),
                              ubar[1:P, ablo:hi, 0:1])
            # ubar[w+1] for w=W-1 is "replicate last": A-mask kills u2 there,
            # but keep the stale zeros in wsu row P-1 (never read: see memset).

            # ======== mu -> z sums (msum1/msum2, unscaled) ========
            for (mlo, mhi) in _blocks(ablo, hi, MB):
                RW = mhi - mlo
                for (mus, msum) in ((mu1, msum1), (mu2, msum2)):
                    csp = ct_.tile([P, MB * C * PROJ], F16, tag="csp",
                                   name="csp")
                    V.tensor_tensor_scan(
                        csp[:, :RW * C * PROJ], flat(pmb[:, :RW]),
                        flat(mus[:, mlo:mhi]), 0.0, op0=OP.mult, op1=OP.add)
                    cs4 = csp[:, :RW * C * PROJ].rearrange(
                        "p (r c j) -> p r c j", r=RW, c=C, j=PROJ)
                    ms = msum[:, mlo:mhi]
                    # msum[z] = sum_{k1<=z} cs[off(k1+1)-1] - cs[off(k1)+z-k1-1]
                    # F part: gather run-total cumulatives T[k1], then a
                    # segmented cumsum over k1 directly into msum.
                    tg = ct_.tile([P, MB, C, L], F16, tag="tg", name="tg")
                    for k1 in range(l):
                        V.tensor_scalar_mul(
                            tg[:, :RW, :, k1:k1 + 1],
                            cs4[:, :, :, off[k1 + 1] - 1:off[k1 + 1]], 1.0)
                    V.tensor_tensor_scan(
                        flat(ms), flat(zmbF[:, :RW]), flat(tg[:, :RW]),
                        0.0, op0=OP.mult, op1=OP.add)
                    for k1 in range(l):
                        z0 = max(k1, 1)
                        a = off[k1] + z0 - k1 - 1
                        V.tensor_tensor(ms[:, :, :, z0:L], ms[:, :, :, z0:L],
                                        cs4[:, :, :, a:a + (L - z0)],
                                        op=OP.subtract)

            # ======== A phase: parabola ========
            for (alo, ahi) in _blocks(ablo, hi, AB):
                R = ahi - alo

                def asl(tl, s=0, e=None):
                    return tl[:, alo + s: ahi + (e or 0)]

                u1 = atile("u1")
                u2 = atile("u2")
                u3 = atile("u3")
                # u1 = (ubar[r+1]-ubar[r]) * A
                V.tensor_tensor(u1[:, :R], ubar[:, alo + 1:ahi + 1],
                                ubar[:, alo:ahi], op=OP.subtract)
                V.tensor_tensor(u1[:, :R], u1[:, :R],
                                bcast_h(mA, alo, ahi, L), op=OP.mult)
                V.tensor_tensor(u1[:, :R], u1[:, :R], msum1[:, alo:ahi],
                                op=OP.add)
                V.scalar_tensor_tensor(u1[:, :R], u1[:, :R], sigmap,
                                       p1[:, alo:ahi], op0=OP.mult, op1=OP.add)
                # u2 = (ubar[w+1]-ubar[w]); w=W-1 -> 0
                if C > 1:
                    V.tensor_tensor(u2[:, :R, 0:C - 1],
                                    ubar[:, alo:ahi, 1:C],
                                    ubar[:, alo:ahi, 0:C - 1], op=OP.subtract)
                V.scalar_tensor_tensor(u2[:, :R, C - 1:C],
                                       ubar[:, alo:ahi, C - 1:C],
                                       wm[:, 1:2], wsu[:, alo:ahi].unsqueeze(2),
                                       op0=OP.mult, op1=OP.add)
                V.tensor_tensor(u2[:, :R], u2[:, :R], msum2[:, alo:ahi],
                                op=OP.add)
                V.scalar_tensor_tensor(u2[:, :R], u2[:, :R], sigmap,
                                       p2[:, alo:ahi], op0=OP.mult, op1=OP.add)
                # u3 = dz(ubar); z=L-1 -> 0
                V.tensor_tensor(u3[:, :R, :, 0:L - 1],
                                ubar[:, alo:ahi, :, 1:L],
                                ubar[:, alo:ahi, :, 0:L - 1], op=OP.subtract)
                V.memset(u3[:, :R, :, L - 1:L], 0.0)
                V.scalar_tensor_tensor(u3[:, :R], u3[:, :R], sigmap,
                                       p3[:, alo:ahi], op0=OP.mult, op1=OP.add)

                # cubic solve
                q2 = atile("q2")
                tq = atile("tq")
                S.activation(q2[:, :R], u1[:, :R], AF.Square)
                S.activation(tq[:, :R], u2[:, :R], AF.Square)
                V.tensor_tensor(q2[:, :R], q2[:, :R], tq[:, :R], op=OP.add)
                bv = atile("dd")
                V.scalar_tensor_tensor(bv[:, :R], q2[:, :R], 0.25,
                                       ld2[:, alo:ahi], op0=OP.mult,
                                       op1=OP.subtract)
                msk = atile("msk", U8)
                V.tensor_tensor(msk[:, :R], u3[:, :R], bv[:, :R], op=OP.is_lt)
                bq = atile("bq")
                V.tensor_tensor(bq[:, :R], u3[:, :R], ld2[:, alo:ahi], op=OP.add)
                S.activation(bq[:, :R], bq[:, :R], AF.Identity,
                             scale=-1.0 / 3.0, bias=2.0 / 3.0)
                b3 = atile("b3")
                V.tensor_tensor(b3[:, :R], bq[:, :R], bq[:, :R], op=OP.mult)
                V.tensor_tensor(b3[:, :R], b3[:, :R], bq[:, :R], op=OP.mult)
                dd = atile("dd")
                V.scalar_tensor_tensor(dd[:, :R], q2[:, :R], 0.25, b3[:, :R],
                                       op0=OP.mult, op1=OP.add)
                dneg = atile("dneg", U8)
                V.tensor_scalar(dneg[:, :R], dd[:, :R], 0.0, None, op0=OP.is_lt)
                norm = atile("norm")
                S.activation(norm[:, :R], q2[:, :R], AF.Sqrt)
                # c = cbrt(0.5*norm + sqrt(max(d,0)))
                sq = atile("sq")
                S.activation(sq[:, :R], dd[:, :R], AF.Relu)
                S.activation(sq[:, :R], sq[:, :R], AF.Sqrt)
                V.scalar_tensor_tensor(sq[:, :R], norm[:, :R], 0.5, sq[:, :R],
                                       op0=OP.mult, op1=OP.add)
                cc = atile("cc")
                S.activation(cc[:, :R], sq[:, :R], AF.Ln)
                S.activation(cc[:, :R], cc[:, :R], AF.Exp, scale=1.0 / 3.0)
                rc = atile("rc")
                V.reciprocal(rc[:, :R], cc[:, :R])
                vv = atile("vv")
                V.tensor_tensor(vv[:, :R], bq[:, :R], rc[:, :R], op=OP.mult)
                V.tensor_tensor(vv[:, :R], cc[:, :R], vv[:, :R], op=OP.subtract)
                # trig branch: v = 2*sb*cos(arccos(ratio)/3), ratio=norm/(2*sb3)
                sb3 = atile("sb3")
                S.activation(sb3[:, :R], b3[:, :R], AF.Relu, scale=-1.0)
                S.activation(sb3[:, :R], sb3[:, :R], AF.Sqrt, scale=4.0)
                V.reciprocal(sb3[:, :R], sb3[:, :R])
                rat = atile("sq")
                V.tensor_tensor(rat[:, :R], norm[:, :R], sb3[:, :R], op=OP.mult)
                V.tensor_scalar(rat[:, :R], rat[:, :R], 0.0, 1.0,
                                op0=OP.max, op1=OP.min)
                # t = sqrt((1-r)/(1+r)); theta = 2*atan(t)
                den = atile("dd")
                V.tensor_scalar(den[:, :R], rat[:, :R], 1.0, None, op0=OP.add)
                V.reciprocal(den[:, :R], den[:, :R])
                V.tensor_scalar(rat[:, :R], rat[:, :R], -1.0, 1.0,
                                op0=OP.mult, op1=OP.add)
                V.tensor_tensor(rat[:, :R], rat[:, :R], den[:, :R], op=OP.mult)
                S.activation(rat[:, :R], rat[:, :R], AF.Sqrt)
                S.activation(rat[:, :R], rat[:, :R], AF.Arctan)
                # v_s = sin(pi/2 - (2/3)atan) = cos(theta/3)
                S.activation(rat[:, :R], rat[:, :R], AF.Sin,
                             scale=-2.0 / 3.0, bias=_HALF_PI)
                sb2 = atile("b3")
                S.activation(sb2[:, :R], bq[:, :R], AF.Relu, scale=-1.0)
                S.activation(sb2[:, :R], sb2[:, :R], AF.Sqrt, scale=4.0)
                V.tensor_tensor(sb2[:, :R], sb2[:, :R], rat[:, :R], op=OP.mult)
                V.copy_predicated(vv[:, :R], dneg[:, :R], sb2[:, :R])
                # scale = 2*v/norm, guarded by norm>0
                V.reciprocal(norm[:, :R], norm[:, :R])
                V.scalar_tensor_tensor(vv[:, :R], vv[:, :R], 2.0, norm[:, :R],
                                       op0=OP.mult, op1=OP.mult)
                nzm = atile("nzm", U8)
                V.tensor_scalar(nzm[:, :R], q2[:, :R], 0.0, None, op0=OP.is_gt)
                V.tensor_tensor(nzm[:, :R], nzm[:, :R], msk[:, :R],
                                op=OP.logical_and)
                # p1,p2 update (in place)
                gu = atile("cc")
                V.tensor_tensor(gu[:, :R], vv[:, :R], u1[:, :R], op=OP.mult)
                S.activation(p1[:, alo:ahi], u1[:, :R], AF.Copy)
                V.copy_predicated(p1[:, alo:ahi], nzm[:, :R], gu[:, :R])
                V.tensor_tensor(gu[:, :R], vv[:, :R], u2[:, :R], op=OP.mult)
                S.activation(p2[:, alo:ahi], u2[:, :R], AF.Copy)
                V.copy_predicated(p2[:, alo:ahi], nzm[:, :R], gu[:, :R])
                # p3 = where(mask, 0.25*(p1n^2+p2n^2) - ld2, u3)
                tq2 = atile("tq")
                S.activation(q2[:, :R], p1[:, alo:ahi], AF.Square)
                S.activation(tq2[:, :R], p2[:, alo:ahi], AF.Square)
                V.tensor_tensor(q2[:, :R], q2[:, :R], tq2[:, :R], op=OP.add)
                V.scalar_tensor_tensor(q2[:, :R], q2[:, :R], 0.25,
                                       ld2[:, alo:ahi], op0=OP.mult,
                                       op1=OP.subtract)
                S.activation(p3[:, alo:ahi], u3[:, :R], AF.Copy)
                V.copy_predicated(p3[:, alo:ahi], msk[:, :R], q2[:, :R])

            # ======== B phase: interval sums, mu update, l2proj ========
            # mu/s are only consumed by the next iteration's A phase, whose
            # row range is [lo, hi-1).
            bhi_all = hi - 1 if NCORES > 1 else hi
            for (blo, bhi) in _blocks(lo, bhi_all, BB):
                R = bhi - blo
                zc1 = bt_.tile([P, BB * C * L], F16, tag="zc1", name="zc1")
                zc2 = bt_.tile([P, BB * C * L], F16, tag="zc2", name="zc2")
                pt = bt_.tile([P, BB, C, L], F16, tag="pt", name="pt")
                for (pn, zc) in ((p1, zc1), (p2, zc2)):
                    V.tensor_scalar_mul(pt[:, :R], pn[:, blo:bhi], tau_mu)
                    V.tensor_tensor_scan(
                        zc[:, :R * C * L], flat(zmb[:, :R]), flat(pt[:, :R]),
                        0.0, op0=OP.mult, op1=OP.add)
                for (sx, mux, zc) in ((s1, mu1, zc1), (s2, mu2, zc2)):
                    zc4 = zc[:, :R * C * L].rearrange(
                        "p (r c z) -> p r c z", r=R, c=C, z=L)
                    # delta = tau*(s - t1): build t1tau into dl then finish
                    dl = btile("dl")
                    for k1 in range(l):
                        # t1tau[p=(k1,k2)] = ics[k2] - ics[k1-1]
                        seg = dl[:, :R, :, off[k1]:off[k1 + 1]]
                        if k1 == 0:
                            S.activation(seg, zc4[:, :, :, 0:L], AF.Copy)
                        else:
                            V.tensor_tensor(
                                seg, zc4[:, :, :, k1:L],
                                zc4[:, :, :, k1 - 1:k1]
                                .broadcast_to([P, R, C, L - k1]),
                                op=OP.subtract)
                    ts_ = btile("tb")
                    V.tensor_scalar_mul(ts_[:, :R], sx[:, blo:bhi], tau_mu)
                    V.tensor_tensor(dl[:, :R], ts_[:, :R], dl[:, :R],
                                    op=OP.subtract)
                    # mu += delta ; m = (s - mu_new) - delta  (= s - mb)
                    V.tensor_tensor(mux[:, blo:bhi], mux[:, blo:bhi],
                                    dl[:, :R], op=OP.add)
                    V.tensor_tensor(sx[:, blo:bhi], sx[:, blo:bhi],
                                    mux[:, blo:bhi], op=OP.subtract)
                    V.tensor_tensor(sx[:, blo:bhi], sx[:, blo:bhi],
                                    dl[:, :R], op=OP.subtract)
                # l2proj: s *= nu / max(|m|, nu)
                n2 = btile("dl")
                tb = btile("tb")
                S.activation(n2[:, :R], s1[:, blo:bhi], AF.Square)
                S.activation(tb[:, :R], s2[:, blo:bhi], AF.Square)
                V.tensor_tensor(n2[:, :R], n2[:, :R], tb[:, :R], op=OP.add)
                S.activation(n2[:, :R], n2[:, :R], AF.Sqrt)
                V.tensor_scalar(n2[:, :R], n2[:, :R], 1.0 / nu, 1.0,
                                op0=OP.mult, op1=OP.max)
                V.reciprocal(n2[:, :R], n2[:, :R])
                V.tensor_tensor(s1[:, blo:bhi], s1[:, blo:bhi], n2[:, :R],
                                op=OP.mult)
                V.tensor_tensor(s2[:, blo:bhi], s2[:, blo:bhi], n2[:, :R],
                                op=OP.mult)

            # ======== C phase: clipping ========
            nc.sync.dma_start(wsp[1:P, lo:hi].unsqueeze(2),
                              p2[0:P - 1, lo:hi, C - 1:C])
            for (alo, ahi) in _blocks(lo, hi, AB):
                R = ahi - alo
                pa = atile("u1")
                pc = atile("u2")
                acc = atile("u3")
                dw = atile("q2")
                # d1 = p1[r]*A[r] - p1[r-1]*C[r-1]
                V.tensor_tensor(pa[:, :R], p1[:, alo:ahi],
                                bcast_h(mA, alo, ahi, L), op=OP.mult)
                V.tensor_tensor(pc[:, :R], p1[:, alo - 1:ahi - 1],
                                bcast_h(mC, alo - 1, ahi - 1, L), op=OP.mult)
                V.tensor_tensor(acc[:, :R], pa[:, :R], pc[:, :R],
                                op=OP.subtract)
                # d2 (w-adjoint): dw[w] = p2[w] - p2[w-1]
                # (wsp[0] == 0 gives the w=0 edge; wA kills p2[W-1] term)
                if C > 1:
                    if C > 2:
                        V.tensor_tensor(dw[:, :R, 1:C - 1],
                                        p2[:, alo:ahi, 1:C - 1],
                                        p2[:, alo:ahi, 0:C - 2], op=OP.subtract)
                    V.scalar_tensor_tensor(dw[:, :R, C - 1:C],
                                           p2[:, alo:ahi, C - 1:C],
                                           wm[:, 0:1],
                                           p2[:, alo:ahi, C - 2:C - 1],
                                           op0=OP.mult, op1=OP.subtract)
                    V.tensor_tensor(dw[:, :R, 0:1], p2[:, alo:ahi, 0:1],
                                    wsp[:, alo:ahi].unsqueeze(2),
                                    op=OP.subtract)
                else:
                    V.scalar_tensor_tensor(dw[:, :R, 0:1],
                                           p2[:, alo:ahi, 0:1], wm[:, 0:1],
                                           wsp[:, alo:ahi].unsqueeze(2),
                                           op0=OP.mult, op1=OP.subtract)
                V.tensor_tensor(acc[:, :R], acc[:, :R], dw[:, :R], op=OP.add)
                # d3 (z-adjoint)
                V.tensor_tensor(dw[:, :R, :, 1:L], p3[:, alo:ahi, :, 1:L],
                                p3[:, alo:ahi, :, 0:L - 1], op=OP.subtract)
                V.tensor_copy(dw[:, :R, :, 0:1], p3[:, alo:ahi, :, 0:1])
                # note: z=L-1 of dw would be -p3[L-2] + p3[L-1] from the sub;
                # true adjoint needs p3eff[L-1]=0 -> overwrite:
                V.tensor_scalar_mul(dw[:, :R, :, L - 1:L],
                                    p3[:, alo:ahi, :, L - 2:L - 1], -1.0)
                V.tensor_tensor(acc[:, :R], acc[:, :R], dw[:, :R], op=OP.add)
                # un = clip(u + tauu*acc); boundary z sets; ubar = 2un - u
                unw = atile("unw", F32)
                V.scalar_tensor_tensor(unw[:, :R], acc[:, :R], tauu,
                                       u[:, alo:ahi], op0=OP.mult, op1=OP.add)
                V.tensor_scalar(unw[:, :R], unw[:, :R], 0.0, 1.0,
                                op0=OP.max, op1=OP.min)
                V.memset(unw[:, :R, :, 0:1], 1.0)
                V.memset(unw[:, :R, :, L - 1:L], 0.0)
                V.scalar_tensor_tensor(ubar[:, alo:ahi], unw[:, :R], 2.0,
                                       u[:, alo:ahi], op0=OP.mult,
                                       op1=OP.subtract)
                S.activation(u[:, alo:ahi], unw[:, :R], AF.Copy)

        # ---------------- output ----------------
        nc.sync.dma_start(u_out.ap(), flat(u[:, G:G + ROWS]))

    nc.compile()
    return nc


_cache = {}


def _get_program(lmbda, nu, repeats, l, cfg_key=None):
    key = (float(lmbda), float(nu), int(repeats), int(l))
    if key not in _cache:
        _cache[key] = build_program(float(lmbda), float(nu), int(repeats),
                                    int(l))
    return _cache[key]


def make_inputs(f, repeats, cfg=None):
    cfg = cfg or CFG
    H, W, L, NCORES, P = cfg["H"], cfg["W"], cfg["L"], cfg["NCORES"], cfg["P"]
    C = W // P
    ROWS = H // NCORES
    G = int(repeats)
    SLAB = ROWS + 2 * G
    f2 = np.asarray(f, dtype=np.float32).reshape(H, W)
    fpad = np.zeros((H + 2 * G, W), np.float32)
    fpad[G:G + H] = f2
    in_maps = []
    for k in range(NCORES):
        slab = fpad[k * ROWS: k * ROWS + SLAB]              # [SLAB, W]
        arr = slab.reshape(SLAB, P, C).transpose(1, 0, 2)   # [P, SLAB, C]
        g = np.arange(SLAB) + k * ROWS - G                  # global row ids
        mAv = ((g >= 0) & (g <= H - 2)).astype(np.float16)
        mCv = ((g >= 0) & (g <= H - 1)).astype(np.float16)
        wmv = np.ones((P, 2), np.float32)
        wmv[:, 1] = -1.0
        wmv[P - 1, :] = 0.0
        in_maps.append({
            "f_in": np.ascontiguousarray(arr.reshape(P, SLAB * C)),
            "mA_in": np.ascontiguousarray(np.broadcast_to(mAv, (P, SLAB))),
            "mC_in": np.ascontiguousarray(np.broadcast_to(mCv, (P, SLAB))),
            "wm_in": wmv,
        })
    return in_maps


def assemble_output(results, repeats, cfg=None):
    cfg = cfg or CFG
    H, W, L, NCORES, P = cfg["H"], cfg["W"], cfg["L"], cfg["NCORES"], cfg["P"]
    C = W // P
    ROWS = H // NCORES
    out = np.empty((H, W, 1, L), np.float32)
    for k in range(NCORES):
        o = results[k]["u_out"].reshape(P, ROWS, C, L)
        out[k * ROWS:(k + 1) * ROWS, :, 0, :] = (
            o.transpose(1, 0, 2, 3).reshape(ROWS, W, L))
    return out


def kernel(f, lmbda, nu, repeats, l):
    l = int(l)
    repeats = int(repeats)
    cfg = dict(CFG)
    cfg["L"] = l
    key = (float(lmbda), float(nu), repeats, l)
    if key not in _cache:
        _cache[key] = build_program(float(lmbda), float(nu), repeats, l,
                                    cfg=cfg)
    nc = _cache[key]
    in_maps = make_inputs(np.asarray(f, np.float32), repeats, cfg=cfg)
    res = run_bass_kernel_spmd(nc, in_maps,
                               core_ids=list(range(cfg["NCORES"])))
    return assemble_output(res.results, repeats, cfg=cfg)


# revision 17
# speedup vs baseline: 442.1386x; 1.0395x over previous
"""Trainium2 Bass kernel for nn_PrimalDual (primal-dual multi-label segmentation).

Strategy:
  - Shard the image rows (h) across 8 cores; each core owns ROWS=48 output rows
    plus G=repeats ghost rows on each side computed redundantly, so no
    inter-core communication is needed (the ghost region shrinks by one row per
    iteration and is exactly exhausted after `repeats` iterations).
  - All state lives in SBUF for the whole solve: u (f32), ubar/p1/p2/p3 (f16),
    s1/s2/mu1/mu2 (f16, the proj=78-sized dual variables).
  - Layout: partition q in [0,128) holds image columns w = C*q + c, c in [0,C),
    C = W/128; free dims are (h_local, c, z|proj).
  - The einsum mu->z and the interval sums z->proj are done with segmented
    scans (tensor_tensor_scan) plus grouped strided subtract ops; everything
    else is pointwise chains on DVE/ACT.
"""

import numpy as np
from contextlib import ExitStack

import concourse.bass as bass
import concourse.tile as tile
from concourse import bacc, mybir
from concourse.bass_utils import run_bass_kernel_spmd

F16 = mybir.dt.float16
U8 = mybir.dt.uint8
F32 = mybir.dt.float32
AF = mybir.ActivationFunctionType
OP = mybir.AluOpType

# problem geometry (from spec; patchable for small-config sim tests)
CFG = dict(H=384, W=384, L=12, NCORES=8, P=128)

AB = 8    # A/C-phase row-block
BB = 10   # B-phase row-block
MB = 20   # mu-sum scan/fold row-block

_HALF_PI = 1.5707963267948966


def flat(ap):
    nd = len(ap.shape)
    if nd == 2:
        return ap
    names = " ".join(f"d{i}" for i in range(nd - 1))
    return ap.rearrange(f"p {names} -> p ({names})")


def _register_consts(nc, values):
    for v in values:
        v = float(v)
        if (mybir.dt.float32, v) in nc.const_aps.aps:
            continue
        t = nc.alloc_sbuf_tensor(f"constf32-{len(nc.const_aps.aps)}", [128, 1], F32)
        nc.gpsimd.memset(t.ap(), v)
        nc.const_aps.aps[(mybir.dt.float32, v)] = t.ap()
    nc.all_engine_barrier()


def _blocks(lo, hi, step):
    out = []
    r = lo
    while r < hi:
        out.append((r, min(r + step, hi)))
        r = out[-1][1]
    return out


def build_program(lmbda, nu, repeats, l, cfg=None):
    cfg = cfg or CFG
    H, W, L, NCORES, P = cfg["H"], cfg["W"], cfg["L"], cfg["NCORES"], cfg["P"]
    assert L == l
    assert W % P == 0
    C = W // P
    ROWS = H // NCORES
    G = repeats
    SLAB = ROWS + 2 * G
    PROJ = l * (l + 1) // 2

    sigmap = 1.0 / (3.0 + l)
    tauu = 1.0 / 6.0
    tau_mu = 1.0 / (2.0 + PROJ / 4.0)
    lmbda = float(lmbda)
    nu = float(nu)
    sql = float(np.sqrt(lmbda))
    kl = [(z + 1) / l for z in range(l)]

    # run offsets: off(k1) = start index of the k1-run in p-order (k1-major)
    off = [0] * (l + 1)
    for k1 in range(l):
        off[k1 + 1] = off[k1] + (l - k1)

    nc = bacc.Bacc("TRN2", target_bir_lowering=False, debug=False,
                   num_devices=NCORES)
    _register_consts(nc, [sql * k for k in kl] + [2.0 / 3.0, _HALF_PI])

    f_in = nc.dram_tensor("f_in", [P, SLAB * C], F32, kind="ExternalInput")
    mA_in = nc.dram_tensor("mA_in", [P, SLAB], F16, kind="ExternalInput")
    mC_in = nc.dram_tensor("mC_in", [P, SLAB], F16, kind="ExternalInput")
    wm_in = nc.dram_tensor("wm_in", [P, 2], F32, kind="ExternalInput")
    u_out = nc.dram_tensor("u_out", [P, ROWS * C * L], F32, kind="ExternalOutput")

    with tile.TileContext(nc) as tc, ExitStack() as ctx, \
            nc.allow_low_precision(reason="f16 state by design"):
        V = nc.vector
        S = nc.scalar

        st = ctx.enter_context(tc.tile_pool(name="state", bufs=1))
        u = st.tile([P, SLAB, C, L], F32)
        ubar = st.tile([P, SLAB, C, L], F16)
        p1 = st.tile([P, SLAB, C, L], F16)
        p2 = st.tile([P, SLAB, C, L], F16)
        p3 = st.tile([P, SLAB, C, L], F16)
        s1 = st.tile([P, SLAB, C, PROJ], F16)
        s2 = st.tile([P, SLAB, C, PROJ], F16)
        mu1 = st.tile([P, SLAB, C, PROJ], F16)
        mu2 = st.tile([P, SLAB, C, PROJ], F16)
        ld2 = st.tile([P, SLAB, C, L], F16)
        fsb = st.tile([P, SLAB, C], F32)
        mA = st.tile([P, SLAB], F16)
        mC = st.tile([P, SLAB], F16)
        zmb = st.tile([P, BB, C, L], F16)     # z-segment mask block (0 at z=0)
        zmbF = st.tile([P, MB, C, L], F16)    # z-segment mask (msum scan)
        pmb = st.tile([P, MB, C, PROJ], F16)  # proj-segment mask block
        msum1 = st.tile([P, SLAB, C, L], F16)  # mu1 -> z sums (unscaled)
        msum2 = st.tile([P, SLAB, C, L], F16)
        # w-shift staging (cross-partition neighbours via DMA)
        wm = st.tile([P, 2], F32)             # [wA, -wA] per-partition
        wsu = st.tile([P, SLAB, L], F16)      # ubar[q+1, c=0] staged at q
        wsp = st.tile([P, SLAB, L], F16)      # p2[q-1, c=C-1] staged at q

        at_ = ctx.enter_context(tc.tile_pool(name="atemp", bufs=2))
        bt_ = ctx.enter_context(tc.tile_pool(name="btemp", bufs=1))
        ct_ = ctx.enter_context(tc.tile_pool(name="csppool", bufs=1))

        def atile(tag, dt=F16):
            return at_.tile([P, AB, C, L], dt, tag=tag, name=tag)

        def btile(tag, dt=F16):
            return bt_.tile([P, BB, C, PROJ], dt, tag=tag, name=tag)

        def bcast_h(m, lo, hi, last):
            return m[:, lo:hi].unsqueeze(2).unsqueeze(3).broadcast_to(
                [P, hi - lo, C, last])

        # ---------------- init ----------------
        nc.sync.dma_start(flat(fsb[:]), f_in.ap())
        nc.sync.dma_start(mA[:], mA_in.ap())
        nc.sync.dma_start(mC[:], mC_in.ap())
        nc.sync.dma_start(wm[:], wm_in.ap())
        fb = fsb[:].unsqueeze(3).broadcast_to([P, SLAB, C, L])
        V.tensor_copy(u[:], fb)
        V.tensor_copy(ubar[:], fb)
        for z in range(L):
            S.activation(ld2[:, :, :, z:z + 1], fsb[:].unsqueeze(3),
                         AF.Square, scale=-sql, bias=sql * kl[z])
        for t in (p1, p2, p3, s1, s2, mu1, mu2):
            nc.gpsimd.memset(t[:], 0.0)
        V.memset(zmb[:], 1.0)
        V.memset(zmb[:, :, :, 0:1], 0.0)
        V.memset(zmbF[:], 1.0)
        V.memset(zmbF[:, :, :, 0:1], 0.0)
        V.memset(pmb[:], 1.0)
        V.memset(pmb[:, :, :, 0:1], 0.0)
        V.memset(wsu[:], 0.0)
        V.memset(wsp[:], 0.0)

        # ---------------- iterations ----------------
        for it in range(repeats):
            lo, hi = it + 1, SLAB - 1 - it
            if NCORES == 1:
                lo, hi = G, G + ROWS  # no ghost shrink needed, masks do edges
            # A/B phases need one extra row above: clipping at row r consumes
            # the same-iteration parabola output at r-1.
            ablo = max(lo - 1, 0)

            # stage w-neighbours for the whole row range
            nc.sync.dma_start(wsu[0:P - 1, ablo:hi].unsqueeze(2),
                              ubar[1:P, ablo:hi, 0:1])
            # ubar[w+1] for w=W-1 is "replicate last": A-mask kills u2 there,
            # but keep the stale zeros in wsu row P-1 (never read: see memset).

            # ======== mu -> z sums (msum1/msum2, unscaled) ========
            # (mu == 0 at iteration 0: skip the whole pipeline)
            for (mlo, mhi) in ([] if it == 0 else _blocks(ablo, hi, MB)):
                RW = mhi - mlo
                for (mus, msum) in ((mu1, msum1), (mu2, msum2)):
                    csp = ct_.tile([P, MB * C * PROJ], F16, tag="csp",
                                   name="csp")
                    V.tensor_tensor_scan(
                        csp[:, :RW * C * PROJ], flat(pmb[:, :RW]),
                        flat(mus[:, mlo:mhi]), 0.0, op0=OP.mult, op1=OP.add)
                    cs4 = csp[:, :RW * C * PROJ].rearrange(
                        "p (r c j) -> p r c j", r=RW, c=C, j=PROJ)
                    ms = msum[:, mlo:mhi]
                    # msum[z] = sum_{k1<=z} cs[off(k1+1)-1] - cs[off(k1)+z-k1-1]
                    # F part: gather run-total cumulatives T[k1], then a
                    # segmented cumsum over k1 directly into msum.
                    tg = ct_.tile([P, MB, C, L], F16, tag="tg", name="tg")
                    for k1 in range(l):
                        V.tensor_scalar_mul(
                            tg[:, :RW, :, k1:k1 + 1],
                            cs4[:, :, :, off[k1 + 1] - 1:off[k1 + 1]], 1.0)
                    V.tensor_tensor_scan(
                        flat(ms), flat(zmbF[:, :RW]), flat(tg[:, :RW]),
                        0.0, op0=OP.mult, op1=OP.add)
                    for k1 in range(l):
                        z0 = max(k1, 1)
                        a = off[k1] + z0 - k1 - 1
                        V.tensor_tensor(ms[:, :, :, z0:L], ms[:, :, :, z0:L],
                                        cs4[:, :, :, a:a + (L - z0)],
                                        op=OP.subtract)

            # ======== A phase: parabola ========
            for (alo, ahi) in _blocks(ablo, hi, AB):
                R = ahi - alo

                def asl(tl, s=0, e=None):
                    return tl[:, alo + s: ahi + (e or 0)]

                u1 = atile("u1")
                u2 = atile("u2")
                u3 = atile("u3")
                # u1 = (ubar[r+1]-ubar[r]) * A
                V.tensor_tensor(u1[:, :R], ubar[:, alo + 1:ahi + 1],
                                ubar[:, alo:ahi], op=OP.subtract)
                V.tensor_tensor(u1[:, :R], u1[:, :R],
                                bcast_h(mA, alo, ahi, L), op=OP.mult)
                if it > 0:
                    V.tensor_tensor(u1[:, :R], u1[:, :R], msum1[:, alo:ahi],
                                    op=OP.add)
                V.scalar_tensor_tensor(u1[:, :R], u1[:, :R], sigmap,
                                       p1[:, alo:ahi], op0=OP.mult, op1=OP.add)
                # u2 = (ubar[w+1]-ubar[w]); w=W-1 -> 0
                if C > 1:
                    V.tensor_tensor(u2[:, :R, 0:C - 1],
                                    ubar[:, alo:ahi, 1:C],
                                    ubar[:, alo:ahi, 0:C - 1], op=OP.subtract)
                V.scalar_tensor_tensor(u2[:, :R, C - 1:C],
                                       ubar[:, alo:ahi, C - 1:C],
                                       wm[:, 1:2], wsu[:, alo:ahi].unsqueeze(2),
                                       op0=OP.mult, op1=OP.add)
                if it > 0:
                    V.tensor_tensor(u2[:, :R], u2[:, :R], msum2[:, alo:ahi],
                                    op=OP.add)
                V.scalar_tensor_tensor(u2[:, :R], u2[:, :R], sigmap,
                                       p2[:, alo:ahi], op0=OP.mult, op1=OP.add)
                # u3 = dz(ubar); z=L-1 -> 0
                V.tensor_tensor(u3[:, :R, :, 0:L - 1],
                                ubar[:, alo:ahi, :, 1:L],
                                ubar[:, alo:ahi, :, 0:L - 1], op=OP.subtract)
                V.memset(u3[:, :R, :, L - 1:L], 0.0)
                V.scalar_tensor_tensor(u3[:, :R], u3[:, :R], sigmap,
                                       p3[:, alo:ahi], op0=OP.mult, op1=OP.add)

                # cubic solve
                q2 = atile("q2")
                tq = atile("tq")
                S.activation(q2[:, :R], u1[:, :R], AF.Square)
                S.activation(tq[:, :R], u2[:, :R], AF.Square)
                V.tensor_tensor(q2[:, :R], q2[:, :R], tq[:, :R], op=OP.add)
                bv = atile("dd")
                V.scalar_tensor_tensor(bv[:, :R], q2[:, :R], 0.25,
                                       ld2[:, alo:ahi], op0=OP.mult,
                                       op1=OP.subtract)
                msk = atile("msk", U8)
                V.tensor_tensor(msk[:, :R], u3[:, :R], bv[:, :R], op=OP.is_lt)
                bq = atile("bq")
                V.tensor_tensor(bq[:, :R], u3[:, :R], ld2[:, alo:ahi], op=OP.add)
                S.activation(bq[:, :R], bq[:, :R], AF.Identity,
                             scale=-1.0 / 3.0, bias=2.0 / 3.0)
                b3 = atile("b3")
                S.activation(b3[:, :R], bq[:, :R], AF.Square)
                V.tensor_tensor(b3[:, :R], b3[:, :R], bq[:, :R], op=OP.mult)
                dd = atile("dd")
                V.scalar_tensor_tensor(dd[:, :R], q2[:, :R], 0.25, b3[:, :R],
                                       op0=OP.mult, op1=OP.add)
                dneg = atile("dneg", U8)
                V.tensor_scalar(dneg[:, :R], dd[:, :R], 0.0, None, op0=OP.is_lt)
                norm = atile("norm")
                S.activation(norm[:, :R], q2[:, :R], AF.Sqrt)
                # c = cbrt(0.5*norm + sqrt(max(d,0)))
                sq = atile("sq")
                S.activation(sq[:, :R], dd[:, :R], AF.Relu)
                S.activation(sq[:, :R], sq[:, :R], AF.Sqrt)
                V.scalar_tensor_tensor(sq[:, :R], norm[:, :R], 0.5, sq[:, :R],
                                       op0=OP.mult, op1=OP.add)
                cc = atile("cc")
                S.activation(cc[:, :R], sq[:, :R], AF.Ln)
                S.activation(cc[:, :R], cc[:, :R], AF.Exp, scale=1.0 / 3.0)
                rc = atile("rc")
                V.reciprocal(rc[:, :R], cc[:, :R])
                vv = atile("vv")
                V.tensor_tensor(vv[:, :R], bq[:, :R], rc[:, :R], op=OP.mult)
                V.tensor_tensor(vv[:, :R], cc[:, :R], vv[:, :R], op=OP.subtract)
                # trig branch: v = 2*sb*cos(arccos(ratio)/3), ratio=norm/(2*sb3)
                sb3 = atile("sb3")
                S.activation(sb3[:, :R], b3[:, :R], AF.Relu, scale=-1.0)
                S.activation(sb3[:, :R], sb3[:, :R], AF.Sqrt, scale=4.0)
                V.reciprocal(sb3[:, :R], sb3[:, :R])
                rat = atile("sq")
                V.tensor_tensor(rat[:, :R], norm[:, :R], sb3[:, :R], op=OP.mult)
                V.tensor_scalar(rat[:, :R], rat[:, :R], 0.0, 1.0,
                                op0=OP.max, op1=OP.min)
                # t = sqrt((1-r)/(1+r)); theta = 2*atan(t)
                den = atile("dd")
                V.tensor_scalar(den[:, :R], rat[:, :R], 1.0, None, op0=OP.add)
                V.reciprocal(den[:, :R], den[:, :R])
                V.tensor_scalar(rat[:, :R], rat[:, :R], -1.0, 1.0,
                                op0=OP.mult, op1=OP.add)
                V.tensor_tensor(rat[:, :R], rat[:, :R], den[:, :R], op=OP.mult)
                S.activation(rat[:, :R], rat[:, :R], AF.Sqrt)
                S.activation(rat[:, :R], rat[:, :R], AF.Arctan)
                # v_s = sin(pi/2 - (2/3)atan) = cos(theta/3)
                S.activation(rat[:, :R], rat[:, :R], AF.Sin,
                             scale=-2.0 / 3.0, bias=_HALF_PI)
                sb2 = atile("b3")
                S.activation(sb2[:, :R], bq[:, :R], AF.Relu, scale=-1.0)
                S.activation(sb2[:, :R], sb2[:, :R], AF.Sqrt, scale=4.0)
                V.tensor_tensor(sb2[:, :R], sb2[:, :R], rat[:, :R], op=OP.mult)
                V.copy_predicated(vv[:, :R], dneg[:, :R], sb2[:, :R])
                # scale = 2*v/norm, guarded by norm>0
                V.reciprocal(norm[:, :R], norm[:, :R])
                V.scalar_tensor_tensor(vv[:, :R], vv[:, :R], 2.0, norm[:, :R],
                                       op0=OP.mult, op1=OP.mult)
                nzm = atile("nzm", U8)
                V.tensor_scalar(nzm[:, :R], q2[:, :R], 0.0, None, op0=OP.is_gt)
                V.tensor_tensor(nzm[:, :R], nzm[:, :R], msk[:, :R],
                                op=OP.logical_and)
                # p1,p2 update (in place)
                gu = atile("cc")
                V.tensor_tensor(gu[:, :R], vv[:, :R], u1[:, :R], op=OP.mult)
                S.activation(p1[:, alo:ahi], u1[:, :R], AF.Copy)
                V.copy_predicated(p1[:, alo:ahi], nzm[:, :R], gu[:, :R])
                V.tensor_tensor(gu[:, :R], vv[:, :R], u2[:, :R], op=OP.mult)
                S.activation(p2[:, alo:ahi], u2[:, :R], AF.Copy)
                V.copy_predicated(p2[:, alo:ahi], nzm[:, :R], gu[:, :R])
                # p3 = where(mask, 0.25*(p1n^2+p2n^2) - ld2, u3)
                tq2 = atile("tq")
                S.activation(q2[:, :R], p1[:, alo:ahi], AF.Square)
                S.activation(tq2[:, :R], p2[:, alo:ahi], AF.Square)
                V.tensor_tensor(q2[:, :R], q2[:, :R], tq2[:, :R], op=OP.add)
                V.scalar_tensor_tensor(q2[:, :R], q2[:, :R], 0.25,
                                       ld2[:, alo:ahi], op0=OP.mult,
                                       op1=OP.subtract)
                S.activation(p3[:, alo:ahi], u3[:, :R], AF.Copy)
                V.copy_predicated(p3[:, alo:ahi], msk[:, :R], q2[:, :R])

            # ======== B phase: interval sums, mu update, l2proj ========
            # mu/s are only consumed by the next iteration's A phase, whose
            # row range is [lo, hi-1).
            bhi_all = hi - 1 if NCORES > 1 else hi
            for (blo, bhi) in _blocks(lo, bhi_all, BB):
                R = bhi - blo
                zc1 = bt_.tile([P, BB * C * L], F16, tag="zc1", name="zc1")
                zc2 = bt_.tile([P, BB * C * L], F16, tag="zc2", name="zc2")
                pt = bt_.tile([P, BB, C, L], F16, tag="pt", name="pt")
                for (pn, zc) in ((p1, zc1), (p2, zc2)):
                    V.tensor_scalar_mul(pt[:, :R], pn[:, blo:bhi], tau_mu)
                    V.tensor_tensor_scan(
                        zc[:, :R * C * L], flat(zmb[:, :R]), flat(pt[:, :R]),
                        0.0, op0=OP.mult, op1=OP.add)
                for (sx, mux, zc) in ((s1, mu1, zc1), (s2, mu2, zc2)):
                    zc4 = zc[:, :R * C * L].rearrange(
                        "p (r c z) -> p r c z", r=R, c=C, z=L)
                    # delta = tau*(s - t1): build t1tau into dl then finish
                    dl = btile("dl")
                    for k1 in range(l):
                        # t1tau[p=(k1,k2)] = ics[k2] - ics[k1-1]
                        seg = dl[:, :R, :, off[k1]:off[k1 + 1]]
                        if k1 == 0:
                            S.activation(seg, zc4[:, :, :, 0:L], AF.Copy)
                        else:
                            V.tensor_tensor(
                                seg, zc4[:, :, :, k1:L],
                                zc4[:, :, :, k1 - 1:k1]
                                .broadcast_to([P, R, C, L - k1]),
                                op=OP.subtract)
                    ts_ = btile("tb")
                    V.tensor_scalar_mul(ts_[:, :R], sx[:, blo:bhi], tau_mu)
                    V.tensor_tensor(dl[:, :R], ts_[:, :R], dl[:, :R],
                                    op=OP.subtract)
                    # mu += delta ; m = (s - mu_new) - delta  (= s - mb)
                    V.tensor_tensor(mux[:, blo:bhi], mux[:, blo:bhi],
                                    dl[:, :R], op=OP.add)
                    V.tensor_tensor(sx[:, blo:bhi], sx[:, blo:bhi],
                                    mux[:, blo:bhi], op=OP.subtract)
                    V.tensor_tensor(sx[:, blo:bhi], sx[:, blo:bhi],
                                    dl[:, :R], op=OP.subtract)
                # l2proj: s *= nu / max(|m|, nu)
                n2 = btile("dl")
                tb = btile("tb")
                S.activation(n2[:, :R], s1[:, blo:bhi], AF.Square)
                S.activation(tb[:, :R], s2[:, blo:bhi], AF.Square)
                V.tensor_tensor(n2[:, :R], n2[:, :R], tb[:, :R], op=OP.add)
                S.activation(n2[:, :R], n2[:, :R], AF.Sqrt)
                V.tensor_scalar(n2[:, :R], n2[:, :R], 1.0 / nu, 1.0,
                                op0=OP.mult, op1=OP.max)
                V.reciprocal(n2[:, :R], n2[:, :R])
                V.tensor_tensor(s1[:, blo:bhi], s1[:, blo:bhi], n2[:, :R],
                                op=OP.mult)
                V.tensor_tensor(s2[:, blo:bhi], s2[:, blo:bhi], n2[:, :R],
                                op=OP.mult)

            # ======== C phase: clipping ========
            nc.sync.dma_start(wsp[1:P, lo:hi].unsqueeze(2),
                              p2[0:P - 1, lo:hi, C - 1:C])
            for (alo, ahi) in _blocks(lo, hi, AB):
                R = ahi - alo
                pa = atile("u1")
                pc = atile("u2")
                acc = atile("u3")
                dw = atile("q2")
                # d1 = p1[r]*A[r] - p1[r-1]*C[r-1]
                V.tensor_tensor(pa[:, :R], p1[:, alo:ahi],
                                bcast_h(mA, alo, ahi, L), op=OP.mult)
                V.tensor_tensor(pc[:, :R], p1[:, alo - 1:ahi - 1],
                                bcast_h(mC, alo - 1, ahi - 1, L), op=OP.mult)
                V.tensor_tensor(acc[:, :R], pa[:, :R], pc[:, :R],
                                op=OP.subtract)
                # d2 (w-adjoint): dw[w] = p2[w] - p2[w-1]
                # (wsp[0] == 0 gives the w=0 edge; wA kills p2[W-1] term)
                if C > 1:
                    if C > 2:
                        V.tensor_tensor(dw[:, :R, 1:C - 1],
                                        p2[:, alo:ahi, 1:C - 1],
                                        p2[:, alo:ahi, 0:C - 2], op=OP.subtract)
                    V.scalar_tensor_tensor(dw[:, :R, C - 1:C],
                                           p2[:, alo:ahi, C - 1:C],
                                           wm[:, 0:1],
                                           p2[:, alo:ahi, C - 2:C - 1],
                                           op0=OP.mult, op1=OP.subtract)
                    V.tensor_tensor(dw[:, :R, 0:1], p2[:, alo:ahi, 0:1],
                                    wsp[:, alo:ahi].unsqueeze(2),
                                    op=OP.subtract)
                else:
                    V.scalar_tensor_tensor(dw[:, :R, 0:1],
                                           p2[:, alo:ahi, 0:1], wm[:, 0:1],
                                           wsp[:, alo:ahi].unsqueeze(2),
                                           op0=OP.mult, op1=OP.subtract)
                V.tensor_tensor(acc[:, :R], acc[:, :R], dw[:, :R], op=OP.add)
                # d3 (z-adjoint)
                V.tensor_tensor(dw[:, :R, :, 1:L], p3[:, alo:ahi, :, 1:L],
                                p3[:, alo:ahi, :, 0:L - 1], op=OP.subtract)
                V.tensor_copy(dw[:, :R, :, 0:1], p3[:, alo:ahi, :, 0:1])
                # note: z=L-1 of dw would be -p3[L-2] + p3[L-1] from the sub;
                # true adjoint needs p3eff[L-1]=0 -> overwrite:
                V.tensor_scalar_mul(dw[:, :R, :, L - 1:L],
                                    p3[:, alo:ahi, :, L - 2:L - 1], -1.0)
                V.tensor_tensor(acc[:, :R], acc[:, :R], dw[:, :R], op=OP.add)
                # un = clip(u + tauu*acc); boundary z sets; ubar = 2un - u
                unw = atile("unw", F32)
                V.scalar_tensor_tensor(unw[:, :R], acc[:, :R], tauu,
                                       u[:, alo:ahi], op0=OP.mult, op1=OP.add)
                V.tensor_scalar(unw[:, :R], unw[:, :R], 0.0, 1.0,
                                op0=OP.max, op1=OP.min)
                V.memset(unw[:, :R, :, 0:1], 1.0)
                V.memset(unw[:, :R, :, L - 1:L], 0.0)
                V.scalar_tensor_tensor(ubar[:, alo:ahi], unw[:, :R], 2.0,
                                       u[:, alo:ahi], op0=OP.mult,
                                       op1=OP.subtract)
                S.activation(u[:, alo:ahi], unw[:, :R], AF.Copy)

        # ---------------- output ----------------
        nc.sync.dma_start(u_out.ap(), flat(u[:, G:G + ROWS]))

    nc.compile()
    return nc


_cache = {}


def _get_program(lmbda, nu, repeats, l, cfg_key=None):
    key = (float(lmbda), float(nu), int(repeats), int(l))
    if key not in _cache:
        _cache[key] = build_program(float(lmbda), float(nu), int(repeats),
                                    int(l))
    return _cache[key]


def make_inputs(f, repeats, cfg=None):
    cfg = cfg or CFG
    H, W, L, NCORES, P = cfg["H"], cfg["W"], cfg["L"], cfg["NCORES"], cfg["P"]
    C = W // P
    ROWS = H // NCORES
    G = int(repeats)
    SLAB = ROWS + 2 * G
    f2 = np.asarray(f, dtype=np.float32).reshape(H, W)
    fpad = np.zeros((H + 2 * G, W), np.float32)
    fpad[G:G + H] = f2
    in_maps = []
    for k in range(NCORES):
        slab = fpad[k * ROWS: k * ROWS + SLAB]              # [SLAB, W]
        arr = slab.reshape(SLAB, P, C).transpose(1, 0, 2)   # [P, SLAB, C]
        g = np.arange(SLAB) + k * ROWS - G                  # global row ids
        mAv = ((g >= 0) & (g <= H - 2)).astype(np.float16)
        mCv = ((g >= 0) & (g <= H - 1)).astype(np.float16)
        wmv = np.ones((P, 2), np.float32)
        wmv[:, 1] = -1.0
        wmv[P - 1, :] = 0.0
        in_maps.append({
            "f_in": np.ascontiguousarray(arr.reshape(P, SLAB * C)),
            "mA_in": np.ascontiguousarray(np.broadcast_to(mAv, (P, SLAB))),
            "mC_in": np.ascontiguousarray(np.broadcast_to(mCv, (P, SLAB))),
            "wm_in": wmv,
        })
    return in_maps


def assemble_output(results, repeats, cfg=None):
    cfg = cfg or CFG
    H, W, L, NCORES, P = cfg["H"], cfg["W"], cfg["L"], cfg["NCORES"], cfg["P"]
    C = W // P
    ROWS = H // NCORES
    out = np.empty((H, W, 1, L), np.float32)
    for k in range(NCORES):
        o = results[k]["u_out"].reshape(P, ROWS, C, L)
        out[k * ROWS:(k + 1) * ROWS, :, 0, :] = (
            o.transpose(1, 0, 2, 3).reshape(ROWS, W, L))
    return out


def kernel(f, lmbda, nu, repeats, l):
    l = int(l)
    repeats = int(repeats)
    cfg = dict(CFG)
    cfg["L"] = l
    key = (float(lmbda), float(nu), repeats, l)
    if key not in _cache:
        _cache[key] = build_program(float(lmbda), float(nu), repeats, l,
                                    cfg=cfg)
    nc = _cache[key]
    in_maps = make_inputs(np.asarray(f, np.float32), repeats, cfg=cfg)
    res = run_bass_kernel_spmd(nc, in_maps,
                               core_ids=list(range(cfg["NCORES"])))
    return assemble_output(res.results, repeats, cfg=cfg)
